# revision 1
# baseline (speedup 1.0000x reference)
"""GAT (2-layer) + global mean pool + MLP + log_softmax on 8 Trainium2 cores.

Strategy (dst-sharded message passing):
  - Nodes partitioned contiguously across 8 cores (12500 each). Each core
    aggregates messages for its own destination nodes.
  - Node feature tables ([h@W | h@W@a_src | h@W@a_dst] per node, f32) are
    computed shard-wise and replicated via AllGather (the graph is random,
    so every core needs ~88% of all nodes anyway - halo == full table).
  - Per-edge h[src] rows are fetched with [128,1]-indirect DMA (one row per
    partition per call), 128 dst-sorted edges at a time.  Segment-softmax
    sums are accumulated with one-hot matmuls on the PE (one-hot built by
    DVE is_equal against a constant iota tile; ad[dst] expanded per edge
    via PE transpose of the one-hot).
  - Graph pooling = one-hot matmul into a PSUM accumulator per block, then
    AllReduce; the tiny MLP + log_softmax run redundantly on every core.
"""

import sys

sys.path.insert(0, "/opt/trn_rl_repo")

import numpy as np

P = 128


def _build_host_data(x, edge_index, batch, W1, a_src1, a_dst1, W2, a_src2, a_dst2,
                     ncores):
    """Pure-integer/graph preprocessing + augmented weights (host side)."""
    N, F_IN = x.shape
    HID = W1.shape[1]
    assert N % ncores == 0
    npc = N // ncores
    nblk = (npc + P - 1) // P
    npc_pad = nblk * P

    src = np.concatenate([edge_index[0], np.arange(N, dtype=np.int64)]).astype(np.int64)
    dst = np.concatenate([edge_index[1], np.arange(N, dtype=np.int64)]).astype(np.int64)

    # padded global row id in the AllGather'd table
    src_pad = (src // npc) * npc_pad + (src % npc)

    core_of = dst // npc
    dst_loc = dst - core_of * npc
    blk_of = dst_loc // P
    dst_rel = dst_loc % P

    # tiles per block: global max so the program is identical on all cores
    tpb = 1
    counts = np.zeros((ncores, nblk), np.int64)
    np.add.at(counts, (core_of, blk_of), 1)
    tpb = int(max(1, np.ceil(counts.max() / P)))

    ncols = nblk * tpb
    idx_all = np.zeros((ncores, P, ncols), np.int32)
    rel_all = np.full((ncores, P, ncols), -1.0, np.float16)
    msk_all = np.full((ncores, P, ncols), -1e5, np.float32)

    order = np.lexsort((dst_loc, core_of))
    so, co, bo, ro, sp = (src[order], core_of[order], blk_of[order],
                         dst_rel[order], src_pad[order])
    # position within (core, block)
    key = co * nblk + bo
    start = np.searchsorted(key, np.arange(ncores * nblk), side="left")
    pos_in_blk = np.arange(len(key)) - start[key]
    col = bo * tpb + pos_in_blk // P
    row = pos_in_blk % P
    idx_all[co, row, col] = sp
    rel_all[co, row, col] = ro.astype(np.float16)
    msk_all[co, row, col] = 0.0

    gid_all = np.full((ncores, P, nblk), -1.0, np.float32)
    for c in range(ncores):
        ids = np.arange(npc) + c * npc
        g = batch[ids].astype(np.float32)
        gg = np.full(npc_pad, -1.0, np.float32)
        gg[:npc] = g
        gid_all[c] = gg.reshape(nblk, P).T

    xT_all = np.zeros((ncores, F_IN, npc_pad), np.float32)
    for c in range(ncores):
        xT_all[c, :, :npc] = x[c * npc:(c + 1) * npc].T

    W1aug = np.concatenate([W1, (W1 @ a_src1)[:, None], (W1 @ a_dst1)[:, None]],
                           axis=1).astype(np.float32)
    W2aug = np.concatenate([W2, (W2 @ a_src2)[:, None], (W2 @ a_dst2)[:, None]],
                           axis=1).astype(np.float32)

    return dict(npc=npc, nblk=nblk, npc_pad=npc_pad, tpb=tpb, ncols=ncols,
                idx_all=idx_all, rel_all=rel_all, msk_all=msk_all,
                gid_all=gid_all, xT_all=xT_all, W1aug=W1aug, W2aug=W2aug)


def _build_program(ncores, nblk, tpb, F_IN, HID, G, C):
    import concourse.bass as bass
    import concourse.bacc as bacc
    import concourse.tile as tile
    from concourse import mybir

    TW = HID + 2          # table row width (h | as | ad)
    ncols = nblk * tpb
    npc_pad = nblk * P
    V = ncores * npc_pad

    nc = bacc.Bacc("TRN2", target_bir_lowering=False, debug=False,
                   num_devices=ncores)
    f32, f16 = mybir.dt.float32, mybir.dt.float16
    Alu = mybir.AluOpType
    Act = mybir.ActivationFunctionType

    ein = lambda n, s, d: nc.dram_tensor(n, s, d, kind="ExternalInput")
    xT_d = ein("xT", [F_IN, npc_pad], f32)
    idx_d = ein("idx", [P, ncols], f32 if False else mybir.dt.int32)
    rel_d = ein("rel", [P, ncols], f16)
    msk_d = ein("msk", [P, ncols], f32)
    gid_d = ein("gid", [P, nblk], f32)
    w1_d = ein("w1aug", [F_IN, TW], f32)
    w2_d = ein("w2aug", [HID, TW], f32)
    b1_d = ein("b1rep", [P, HID], f32)
    b2_d = ein("b2rep", [P, HID], f32)
    lw_d = ein("lin_w", [HID, HID // 2], f32)
    lb_d = ein("lin_b", [HID // 2, 1], f32)
    cw_d = ein("cls_w", [HID // 2, C], f32)
    cb_d = ein("cls_b", [C, 1], f32)
    io16_d = ein("iota16", [P, P], f16)
    id16_d = ein("ident16", [P, P], f16)
    id32_d = ein("ident32", [P, P], f32)
    one16_d = ein("ones16", [P, 1], f16)
    out_d = nc.dram_tensor("out", [G, C], f32, kind="ExternalOutput")

    with tile.TileContext(nc) as tc:
        with (
            tc.tile_pool(name="cst", bufs=1) as cst,
            tc.tile_pool(name="sb", bufs=3) as sb,
            tc.tile_pool(name="ps", bufs=4, space="PSUM") as ps,
            tc.tile_pool(name="psacc", bufs=1, space="PSUM") as psacc,
            tc.tile_pool(name="dram", bufs=1, space="DRAM") as dram,
        ):
            # ---- constants ----
            idx_t = cst.tile([P, ncols], mybir.dt.int32)
            nc.sync.dma_start(idx_t[:], idx_d[:, :])
            rel_t = cst.tile([P, ncols], f16)
            nc.sync.dma_start(rel_t[:], rel_d[:, :])
            msk_t = cst.tile([P, ncols], f32)
            nc.sync.dma_start(msk_t[:], msk_d[:, :])
            gid_t = cst.tile([P, nblk], f32)
            nc.sync.dma_start(gid_t[:], gid_d[:, :])
            w1_t = cst.tile([F_IN, TW], f32)
            nc.sync.dma_start(w1_t[:], w1_d[:, :])
            w2_t = cst.tile([HID, TW], f32)
            nc.sync.dma_start(w2_t[:], w2_d[:, :])
            w2_t16 = cst.tile([HID, TW], f16)
            nc.vector.tensor_copy(w2_t16[:], w2_t[:])
            b1_t = cst.tile([P, HID], f32)
            nc.sync.dma_start(b1_t[:], b1_d[:, :])
            b2_t = cst.tile([P, HID], f32)
            nc.sync.dma_start(b2_t[:], b2_d[:, :])
            io16_t = cst.tile([P, P], f16)
            nc.sync.dma_start(io16_t[:], io16_d[:, :])
            id16_t = cst.tile([P, P], f16)
            nc.sync.dma_start(id16_t[:], id16_d[:, :])
            id32_t = cst.tile([P, P], f32)
            nc.sync.dma_start(id32_t[:], id32_d[:, :])
            one16_t = cst.tile([P, 1], f16)
            nc.sync.dma_start(one16_t[:], one16_d[:, :])
            xT_t = cst.tile([F_IN, npc_pad], f32)
            nc.sync.dma_start(xT_t[:], xT_d[:, :])

            # DRAM: local table slices + replicated tables
            slice1 = dram.tile([npc_pad, TW], f32)
            slice2 = dram.tile([npc_pad, TW], f32)
            table1 = dram.tile([V, TW], f32)
            table2 = dram.tile([V, TW], f32)
            pool_in = dram.tile([P, HID + 1], f32)
            pool_out = dram.tile([P, HID + 1], f32)

            # pooled accumulator (SBUF, across all blocks of layer 2)
            pooled = cst.tile([P, HID + 1], f32)
            nc.vector.memset(pooled[:], 0.0)

            # ---- phase 0: table1 slice = [x@W1 | as1 | ad1] ----
            with tc.For_i(0, nblk, 1) as b:
                xb = sb.tile([F_IN, P], f32, tag="xb")
                nc.vector.tensor_copy(xb[:], xT_t[:, bass.ds(b * P, P)])
                t1T_ps = ps.tile([TW, P], f32, tag="pst")
                nc.tensor.matmul(t1T_ps[:], lhsT=w1_t[:], rhs=xb[:],
                                 start=True, stop=True)
                t1T_sb = sb.tile([TW, P], f16, tag="t1Tsb")
                nc.vector.tensor_copy(t1T_sb[:], t1T_ps[:])
                t1_ps = ps.tile([P, TW], f16, tag="pst")
                nc.tensor.transpose(t1_ps[:], t1T_sb[:], id16_t[:TW, :TW])
                t1_sb = sb.tile([P, TW], f32, tag="t1sb")
                nc.vector.tensor_copy(t1_sb[:], t1_ps[:])
                nc.sync.dma_start(slice1[bass.ds(b * P, P), :], t1_sb[:])

            nc.gpsimd.collective_compute(
                "AllGather", Alu.bypass,
                replica_groups=[list(range(ncores))],
                ins=[slice1.opt()], outs=[table1.opt()],
            )

            def gat_layer(table_ap, slice_ap, out_slice_ap, is_last):
                # per-node ad column for this core: ad[p, b] = slice[b*128+p, TW-1]
                ad_grid = cst.tile([P, nblk], f32,
                                   name=f"adgrid{int(is_last)}")
                nc.sync.dma_start(
                    ad_grid[:],
                    slice_ap[:, TW - 1:TW].rearrange("(b p) c -> p (b c)", p=P),
                )
                with tc.For_i(0, nblk, 1) as b:
                    idx_blk = sb.tile([P, tpb], mybir.dt.int32, tag="idxblk")
                    nc.vector.tensor_copy(idx_blk[:], idx_t[:, bass.ds(b * tpb, tpb)])
                    rel_blk = sb.tile([P, tpb], f16, tag="relblk")
                    nc.vector.tensor_copy(rel_blk[:], rel_t[:, bass.ds(b * tpb, tpb)])
                    ad_blk = sb.tile([P, 1], f16, tag="adblk")
                    nc.vector.tensor_copy(ad_blk[:], ad_grid[:, bass.ds(b, 1)])

                    # AdR[p, v] = ad_blk[v] (transpose of the free-dim bcast)
                    adR_ps = ps.tile([P, P], f16, tag="pst")
                    nc.tensor.transpose(adR_ps[:],
                                        ad_blk[:, 0:1].to_broadcast([P, P]),
                                        id16_t[:])
                    adR = sb.tile([P, P], f32, tag="adR")
                    nc.vector.tensor_copy(adR[:], adR_ps[:])

                    acc = psacc.tile([P, HID + 1], f32, tag="acc")
                    for t in range(tpb):
                        g = sb.tile([P, TW], f32, tag="g")
                        nc.gpsimd.indirect_dma_start(
                            out=g[:], out_offset=None, in_=table_ap[:, :],
                            in_offset=bass.IndirectOffsetOnAxis(
                                ap=idx_blk[:, t:t + 1], axis=0),
                        )
                        S = sb.tile([P, P], f16, tag="S")
                        nc.vector.tensor_tensor(
                            out=S[:],
                            in0=rel_blk[:, t:t + 1].to_broadcast([P, P]),
                            in1=io16_t[:], op=Alu.is_equal)
                        ex = sb.tile([P, P], f32, tag="ex")
                        nc.scalar.activation(ex[:], adR[:], Act.Lrelu,
                                             bias=g[:, HID:HID + 1], alpha=0.2)
                        nc.scalar.activation(ex[:], ex[:], Act.Exp)
                        exS = sb.tile([P, P], f16, tag="exS")
                        nc.vector.tensor_tensor(out=exS[:], in0=ex[:], in1=S[:],
                                                op=Alu.mult)
                        rhs = sb.tile([P, HID + 1], f16, tag="rhs")
                        nc.vector.tensor_copy(rhs[:, 0:HID], g[:, 0:HID])
                        nc.vector.tensor_copy(rhs[:, HID:HID + 1], one16_t[:])
                        nc.tensor.matmul(acc[:], lhsT=exS[:], rhs=rhs[:],
                                         start=(t == 0), stop=(t == tpb - 1))

                    # block epilogue: out_v = num/denom (+bias) [+relu]
                    den = sb.tile([P, 1], f32, tag="den")
                    nc.vector.tensor_scalar(out=den[:], in0=acc[:, HID:HID + 1],
                                            scalar1=1e-30, scalar2=None, op0=Alu.max)
                    rec = sb.tile([P, 1], f32, tag="rec")
                    nc.vector.reciprocal(rec[:], den[:])
                    hv = sb.tile([P, HID], f32, tag="hv")
                    nc.vector.tensor_tensor(out=hv[:], in0=acc[:, 0:HID],
                                            in1=rec[:].to_broadcast([P, HID]),
                                            op=Alu.mult)
                    if not is_last:
                        nc.vector.tensor_tensor(out=hv[:], in0=hv[:], in1=b1_t[:],
                                                op=Alu.add)
                        nc.scalar.activation(hv[:], hv[:], Act.Relu)
                        # table2 row = [hv@W2 | as2 | ad2]
                        hv16 = sb.tile([P, HID], f16, tag="hv16")
                        nc.vector.tensor_copy(hv16[:], hv[:])
                        hvT_ps = ps.tile([HID, P], f16, tag="pst")
                        nc.tensor.transpose(hvT_ps[:], hv16[:], id16_t[:])
                        hvT = sb.tile([HID, P], f16, tag="hvTsb")
                        nc.vector.tensor_copy(hvT[:], hvT_ps[:])
                        t2T_ps = ps.tile([TW, P], f32, tag="pst")
                        nc.tensor.matmul(t2T_ps[:], lhsT=w2_t16[:], rhs=hvT[:],
                                         start=True, stop=True)
                        t2T_sb = sb.tile([TW, P], f16, tag="t2Tsb")
                        nc.vector.tensor_copy(t2T_sb[:], t2T_ps[:])
                        t2_ps = ps.tile([P, TW], f16, tag="pst")
                        nc.tensor.transpose(t2_ps[:], t2T_sb[:], id16_t[:TW, :TW])
                        t2_sb = sb.tile([P, TW], f32, tag="t2sb")
                        nc.vector.tensor_copy(t2_sb[:], t2_ps[:])
                        nc.sync.dma_start(out_slice_ap[bass.ds(b * P, P), :],
                                          t2_sb[:])
                    else:
                        nc.vector.tensor_tensor(out=hv[:], in0=hv[:], in1=b2_t[:],
                                                op=Alu.add)
                        # pooling: pooled += G_onehot^T @ [hv | 1]
                        prhs = sb.tile([P, HID + 1], f16, tag="prhs")
                        nc.vector.tensor_copy(prhs[:, 0:HID], hv[:])
                        nc.vector.tensor_copy(prhs[:, HID:HID + 1], one16_t[:])
                        gid_col = sb.tile([P, 1], f16, tag="gidcol")
                        nc.vector.tensor_copy(gid_col[:], gid_t[:, bass.ds(b, 1)])
                        Gh = sb.tile([P, P], f16, tag="Gh")
                        nc.vector.tensor_tensor(
                            out=Gh[:], in0=gid_col[:].to_broadcast([P, P]),
                            in1=io16_t[:], op=Alu.is_equal)
                        pool_ps = ps.tile([P, HID + 1], f32, tag="pst")
                        nc.tensor.matmul(pool_ps[:], lhsT=Gh[:], rhs=prhs[:],
                                         start=True, stop=True)
                        nc.vector.tensor_tensor(out=pooled[:], in0=pooled[:],
                                                in1=pool_ps[:], op=Alu.add)

            gat_layer(table1, slice1, slice2, is_last=False)
            nc.gpsimd.collective_compute(
                "AllGather", Alu.bypass,
                replica_groups=[list(range(ncores))],
                ins=[slice2.opt()], outs=[table2.opt()],
            )
            gat_layer(table2, slice2, None, is_last=True)

            # ---- AllReduce pooled sums ----
            nc.sync.dma_start(pool_in[:, :], pooled[:])
            nc.gpsimd.collective_compute(
                "AllReduce", Alu.add,
                replica_groups=[list(range(ncores))],
                ins=[pool_in.opt()], outs=[pool_out.opt()],
            )
            pl = sb.tile([P, HID + 1], f32, tag="pl")
            nc.sync.dma_start(pl[:], pool_out[:, :])

            # mean = sum / max(count, 1)
            cnt = sb.tile([P, 1], f32, tag="cnt")
            nc.vector.tensor_scalar(out=cnt[:], in0=pl[:, HID:HID + 1],
                                    scalar1=1.0, scalar2=None, op0=Alu.max)
            crec = sb.tile([P, 1], f32, tag="crec")
            nc.vector.reciprocal(crec[:], cnt[:])
            mean = sb.tile([P, HID], f32, tag="mean")
            nc.vector.tensor_tensor(out=mean[:], in0=pl[:, 0:HID],
                                    in1=crec[:].to_broadcast([P, HID]), op=Alu.mult)

            # MLP: z = relu(mean @ lin_w + lin_b); logits = z @ cls_w + cls_b
            lw_t = cst.tile([HID, HID // 2], f32)
            nc.sync.dma_start(lw_t[:], lw_d[:, :])
            lb_t = cst.tile([HID // 2, 1], f32)
            nc.sync.dma_start(lb_t[:], lb_d[:, :])
            cw_t = cst.tile([HID // 2, C], f32)
            nc.sync.dma_start(cw_t[:], cw_d[:, :])
            cb_t = cst.tile([C, 1], f32)
            nc.sync.dma_start(cb_t[:], cb_d[:, :])

            meanT_ps = ps.tile([HID, P], f32, tag="pst")
            nc.tensor.transpose(meanT_ps[:], mean[:], id32_t[:])
            meanT = sb.tile([HID, P], f32, tag="meanTsb")
            nc.vector.tensor_copy(meanT[:], meanT_ps[:])
            zT_ps = ps.tile([HID // 2, P], f32, tag="pst")
            nc.tensor.matmul(zT_ps[:], lhsT=lw_t[:], rhs=meanT[:],
                             start=True, stop=True)
            zT = sb.tile([HID // 2, P], f32, tag="zTsb")
            nc.scalar.activation(zT[:], zT_ps[:], Act.Relu, bias=lb_t[:])
            lgT_ps = ps.tile([C, P], f32, tag="pst")
            nc.tensor.matmul(lgT_ps[:], lhsT=cw_t[:], rhs=zT[:],
                             start=True, stop=True)
            lgT = sb.tile([C, P], f32, tag="lgTsb")
            nc.scalar.activation(lgT[:], lgT_ps[:], Act.Identity, bias=cb_t[:])
            lg_ps = ps.tile([P, C], f32, tag="pst")
            nc.tensor.transpose(lg_ps[:], lgT[:], id32_t[:C, :C])
            lg = sb.tile([P, C], f32, tag="lgsb")
            nc.vector.tensor_copy(lg[:], lg_ps[:])

            # log_softmax over classes
            mx = sb.tile([P, 1], f32, tag="mx")
            nc.vector.tensor_reduce(mx[:], lg[:], axis=mybir.AxisListType.X,
                                    op=Alu.max)
            sh = sb.tile([P, C], f32, tag="sh")
            nc.vector.tensor_tensor(out=sh[:], in0=lg[:],
                                    in1=mx[:].to_broadcast([P, C]),
                                    op=Alu.subtract)
            exs = sb.tile([P, C], f32, tag="exs")
            se = sb.tile([P, 1], f32, tag="se")
            nc.scalar.activation(exs[:], sh[:], Act.Exp, accum_out=se[:])
            lse = sb.tile([P, 1], f32, tag="lse")
            nc.scalar.activation(lse[:], se[:], Act.Ln)
            res = sb.tile([P, C], f32, tag="res")
            nc.vector.tensor_tensor(out=res[:], in0=sh[:],
                                    in1=lse[:].to_broadcast([P, C]),
                                    op=Alu.subtract)
            nc.sync.dma_start(out_d[:, :], res[:])

    nc.compile()
    return nc


def run_gnn(inputs, ncores=8, trace=False):
    from concourse.bass_utils import run_bass_kernel_spmd

    x = np.asarray(inputs["x"], np.float32)
    edge_index = np.asarray(inputs["edge_index"])
    batch = np.asarray(inputs["batch"])
    W1 = np.asarray(inputs["W1"], np.float32)
    W2 = np.asarray(inputs["W2"], np.float32)
    hd = _build_host_data(
        x, edge_index, batch, W1,
        np.asarray(inputs["a_src1"], np.float32),
        np.asarray(inputs["a_dst1"], np.float32),
        W2,
        np.asarray(inputs["a_src2"], np.float32),
        np.asarray(inputs["a_dst2"], np.float32),
        ncores)

    N, F_IN = x.shape
    HID = W1.shape[1]
    G = 128  # number of graphs == P (pooling one-hot relies on this)
    C = np.asarray(inputs["cls_w"]).shape[1]

    nc = _build_program(ncores, hd["nblk"], hd["tpb"], F_IN, HID, G, C)

    iota16 = np.tile(np.arange(P, dtype=np.float16)[None, :], (P, 1))
    ident16 = np.eye(P, dtype=np.float16)
    ident32 = np.eye(P, dtype=np.float32)
    ones16 = np.ones((P, 1), np.float16)
    b1rep = np.tile(np.asarray(inputs["b1"], np.float32)[None, :], (P, 1))
    b2rep = np.tile(np.asarray(inputs["b2"], np.float32)[None, :], (P, 1))

    in_maps = []
    for c in range(ncores):
        in_maps.append({
            "xT": hd["xT_all"][c],
            "idx": hd["idx_all"][c],
            "rel": hd["rel_all"][c],
            "msk": hd["msk_all"][c],
            "gid": hd["gid_all"][c],
            "w1aug": hd["W1aug"],
            "w2aug": hd["W2aug"],
            "b1rep": b1rep,
            "b2rep": b2rep,
            "lin_w": np.asarray(inputs["lin_w"], np.float32),
            "lin_b": np.asarray(inputs["lin_b"], np.float32)[:, None],
            "cls_w": np.asarray(inputs["cls_w"], np.float32),
            "cls_b": np.asarray(inputs["cls_b"], np.float32)[:, None],
            "iota16": iota16,
            "ident16": ident16,
            "ident32": ident32,
            "ones16": ones16,
        })

    res = run_bass_kernel_spmd(nc, in_maps, core_ids=list(range(ncores)),
                               trace=trace)
    out = res.results[0]["out"]
    return out, res


def kernel(**inputs):
    out, _ = run_gnn(inputs, ncores=8)
    return out.astype(np.float32)



# revision 16
# speedup vs baseline: 1.9664x; 1.9664x over previous
"""GAT (2-layer) + global mean pool + MLP + log_softmax on 8 Trainium2 cores.

Strategy (dst-sharded message passing, bulk-gather edition):
  - Nodes partitioned across 8 cores; per-core node tables
    ([h@W | 1 | as | ad] as 128-col f16 rows = 256B) are computed shard-wise
    and replicated via AllGather into DRAM tables.
  - Per-edge h[src] rows are fetched with Pool-engine dma_gather (<=1024
    indices per call - the ucode limit), one call per (dst-block, quarter),
    rotated over 4 SWDGE queues (descriptor generation parallelizes ~3.6x
    across queues).  The table is addressed in 4 quarters so indices fit
    int16.
  - Local nodes are bin-packed into blocks balancing per-(block, quarter)
    edge counts, so every bucket fits tpq tiles of 128 edges with minimal
    padding.  Self loops bypass the gather entirely (local rows, identity
    one-hot).
  - Per-edge attention weights use exp(leakyrelu(x)) = max(exp(x),
    exp(0.2x)): two Exp passes on the otherwise-idle scalar engine (table
    pinned to Exp), so the vector engine only builds the one-hot, one add
    and one max per block.
  - Aggregation is a PSUM-accumulated one-hot matmul per 128-edge tile with
    the softmax denominator riding along as the table's constant-1 column.
  - Graph pooling = one-hot matmul + AllReduce; tiny MLP + log_softmax run
    redundantly on every core.
"""

import sys

sys.path.insert(0, "/opt/trn_rl_repo")

import numpy as np

P = 128
NQ = 4          # src-quarters (int16 index range per dma_gather)
ROW = 128       # table row width in f16 elems (256B, dma_gather granularity)
HID = 64
ONEC = HID      # constant-1 column (denominator rides the matmul)
ASC = HID + 1   # alpha_src column
ADC = HID + 2   # alpha_dst column
TW = HID + 3    # populated row prefix
RW = HID + 1    # rhs width for the aggregation matmul: [h | 1]
CHUNK = 10      # dst blocks per gather chunk


def _pack_blocks(deg_q, nblk, cap):
    """First-fit-decreasing bin packing: assign nodes (rows of deg_q
    [npc, NQ]) to nblk bins with <= P nodes per bin and per-quarter edge
    count <= cap.  Returns pos[npc] (slot b*P + i) or None."""
    npc = deg_q.shape[0]
    order = np.argsort(-deg_q.sum(1), kind="stable")
    rem = np.full((nblk, NQ), cap, np.int64)
    cnt = np.zeros(nblk, np.int64)
    pos = np.empty(npc, np.int64)
    for l in order:
        ok = (cnt < P) & (rem >= deg_q[l]).all(1)
        b = int(np.argmax(ok))
        if not ok[b]:
            return None
        pos[l] = b * P + cnt[b]
        cnt[b] += 1
        rem[b] -= deg_q[l]
    return pos


def _build_host_data(x, edge_index, batch, W1, a_src1, a_dst1, W2, a_src2,
                     a_dst2, ncores):
    """Pure-integer/graph preprocessing + augmented weights (host side)."""
    N, F_IN = x.shape
    assert N % ncores == 0 and ncores % NQ == 0
    npc = N // ncores
    nblk = -(-npc // P)
    chunk = min(CHUNK, nblk)
    nblk = -(-nblk // chunk) * chunk
    npc_pad = nblk * P
    V = ncores * npc_pad
    qrows = V // NQ
    assert qrows <= 32767
    nchunk = nblk // chunk

    # self loops are handled separately on-device (local rows, no gather)
    src = np.asarray(edge_index[0])
    dst = np.asarray(edge_index[1])
    score = (src // npc).astype(np.int64)
    dcore = (dst // npc).astype(np.int64)
    sloc = (src % npc).astype(np.int64)
    dloc = (dst % npc).astype(np.int64)

    # per-node quarter in-degree, then degree-balanced packing into blocks
    deg = np.zeros((ncores, npc, NQ), np.int64)
    # quarter of the src depends on its packed position; quarters span whole
    # cores (qrows is a multiple of npc_pad * cores-per-quarter), so the
    # quarter is known before packing:
    cpq = ncores // NQ
    q_of = score // cpq
    np.add.at(deg, (dcore, dloc, q_of), 1)

    pos_all = np.empty((ncores, npc), np.int64)
    tpq = None
    for cap_tiles in range(4, 9):
        ok = True
        for c in range(ncores):
            pos = _pack_blocks(deg[c], nblk, cap_tiles * P)
            if pos is None:
                ok = False
                break
            pos_all[c] = pos
        if ok:
            tpq = cap_tiles
            break
    assert tpq is not None
    assert tpq * P <= 1024  # dma_gather ucode faults above 1024 indices
    tpb = NQ * tpq

    srow = score * npc_pad + pos_all[score, sloc]
    qoff = (srow % qrows).astype(np.int64)
    dpos = pos_all[dcore, dloc]
    blk = dpos // P
    rel = dpos % P

    cnt = np.zeros((ncores, nblk, NQ), np.int64)
    np.add.at(cnt, (dcore, blk, q_of), 1)
    tpq = int(max(1, -(-cnt.max() // P)))
    tpb = NQ * tpq

    call_idx = np.zeros((ncores, NQ, nblk, tpq * P), np.int16)
    rel_all = np.full((ncores, P, nblk * tpb), -1.0, np.float16)

    order = np.lexsort((q_of, blk, dcore))
    so_q, so_b, so_c = q_of[order], blk[order], dcore[order]
    so_qoff, so_rel = qoff[order], rel[order]
    key = (so_c * nblk + so_b) * NQ + so_q
    start = np.searchsorted(key, np.arange(ncores * nblk * NQ), side="left")
    pos = np.arange(len(key)) - start[key]
    call_idx[so_c, so_q, so_b, pos] = so_qoff.astype(np.int16)
    rel_col = so_b * tpb + so_q * tpq + pos // P
    rel_all[so_c, pos % P, rel_col] = so_rel.astype(np.float16)

    # wrap indices for the gpsimd cores: idx j lives at [p % 16 == j % 16,
    # j // 16], replicated across the 8 groups of 16 partitions
    ci = call_idx.reshape(ncores, NQ, nblk, -1, 16)
    ci = np.transpose(ci, (0, 1, 2, 4, 3))
    ci = np.broadcast_to(ci[:, :, :, None, :, :],
                         (ncores, NQ, nblk, 8, 16, tpq * P // 16))
    idx_all = ci.reshape(ncores, NQ, nblk, P, -1)
    idx_all = np.transpose(idx_all, (0, 3, 1, 2, 4)).reshape(ncores, P, -1)
    idx_all = np.ascontiguousarray(idx_all)

    gid_all = np.full((ncores, P, nblk), -1.0, np.float32)
    xT_all = np.zeros((ncores, F_IN, npc_pad), np.float16)
    for c in range(ncores):
        ids = np.arange(npc)
        gg = np.full(npc_pad, -1.0, np.float32)
        gg[pos_all[c]] = batch[ids + c * npc].astype(np.float32)
        gid_all[c] = gg.reshape(nblk, P).T
        xT_all[c][:, pos_all[c]] = x[c * npc:(c + 1) * npc].T.astype(
            np.float16)

    def aug(W, a_s, a_d):
        w = np.zeros((W.shape[0], TW), np.float32)
        w[:, :HID] = W
        w[:, ASC] = W @ a_s
        w[:, ADC] = W @ a_d
        return w.astype(np.float16)

    return dict(npc=npc, nblk=nblk, npc_pad=npc_pad, tpq=tpq, tpb=tpb, V=V,
                chunk=chunk, qrows=qrows, nchunk=nchunk, idx_all=idx_all,
                rel_all=rel_all, gid_all=gid_all, xT_all=xT_all,
                W1aug=aug(W1, a_src1, a_dst1), W2aug=aug(W2, a_src2, a_dst2))


def _build_program(ncores, nblk, tpq, F_IN, G, C, V, qrows, nchunk, chunk):
    import concourse.bass as bass
    import concourse.bacc as bacc
    import concourse.tile as tile
    from concourse import mybir

    tpb = NQ * tpq
    npc_pad = nblk * P
    nidx = tpq * P                    # indices per (block, quarter) gather
    idxw = nidx // 16                 # idx cols per call (int16, wrapped)

    nc = bacc.Bacc("TRN2", target_bir_lowering=False, debug=False,
                   num_devices=ncores, num_swdge_queues=4)
    f32, f16, i16 = mybir.dt.float32, mybir.dt.float16, mybir.dt.int16
    Alu = mybir.AluOpType
    Act = mybir.ActivationFunctionType

    ein = lambda n, s, d: nc.dram_tensor(n, s, d, kind="ExternalInput")
    xT_d = ein("xT", [F_IN, npc_pad], f16)
    idx_d = ein("idx", [P, NQ * nblk * idxw], i16)
    rel_d = ein("rel", [P, nblk * tpb], f16)
    gid_d = ein("gid", [P, nblk], f32)
    w1_d = ein("w1aug", [F_IN, TW], f16)
    w2_d = ein("w2aug", [HID, TW], f16)
    b1_d = ein("b1rep", [P, HID], f32)
    b2_d = ein("b2rep", [P, HID], f32)
    lw_d = ein("lin_w", [HID, HID // 2], f32)
    lb_d = ein("lin_b", [HID // 2, 1], f32)
    cw_d = ein("cls_w", [HID // 2, C], f32)
    cb_d = ein("cls_b", [C, 1], f32)
    io16_d = ein("iota16", [P, P], f16)
    id16_d = ein("ident16", [P, P], f16)
    id32_d = ein("ident32", [P, P], f32)
    one16_d = ein("ones16", [P, 1], f16)
    out_d = nc.dram_tensor("out", [G, C], f32, kind="ExternalOutput")

    table1 = nc.dram_tensor("table1", [V, ROW], f16, kind="Internal")
    table2 = nc.dram_tensor("table2", [V, ROW], f16, kind="Internal")

    with tile.TileContext(nc) as tc:
        with (
            tc.tile_pool(name="cst", bufs=1) as cst,
            tc.tile_pool(name="sb", bufs=3) as sb,
            tc.tile_pool(name="gat", bufs=2) as gat,
            tc.tile_pool(name="ps", bufs=4, space="PSUM") as ps,
            tc.tile_pool(name="psacc", bufs=4, space="PSUM") as psacc,
            tc.tile_pool(name="dram", bufs=1, space="DRAM") as dram,
        ):
            # ---- constants ----
            rel_t = cst.tile([P, nblk * tpb], f16)
            nc.sync.dma_start(rel_t[:], rel_d[:, :])
            gid_t = cst.tile([P, nblk], f32)
            nc.sync.dma_start(gid_t[:], gid_d[:, :])
            w1_t = cst.tile([F_IN, TW], f16)
            nc.sync.dma_start(w1_t[:], w1_d[:, :])
            w2_t = cst.tile([HID, TW], f16)
            nc.sync.dma_start(w2_t[:], w2_d[:, :])
            b1_t = cst.tile([P, HID], f32)
            nc.sync.dma_start(b1_t[:], b1_d[:, :])
            b2_t = cst.tile([P, HID], f32)
            nc.sync.dma_start(b2_t[:], b2_d[:, :])
            io16_t = cst.tile([P, P], f16)
            nc.sync.dma_start(io16_t[:], io16_d[:, :])
            id16_t = cst.tile([P, P], f16)
            nc.sync.dma_start(id16_t[:], id16_d[:, :])
            id32_t = cst.tile([P, P], f32)
            nc.sync.dma_start(id32_t[:], id32_d[:, :])
            one16_t = cst.tile([P, 1], f16)
            nc.sync.dma_start(one16_t[:], one16_d[:, :])
            xT_t = cst.tile([F_IN, npc_pad], f16)
            nc.sync.dma_start(xT_t[:], xT_d[:, :])

            slice1 = dram.tile([npc_pad, ROW], f16)
            slice2 = dram.tile([npc_pad, ROW], f16)
            pool_in = dram.tile([P, HID + 1], f32)
            pool_out = dram.tile([P, HID + 1], f32)

            pooled = cst.tile([P, HID + 1], f32)
            nc.vector.memset(pooled[:], 0.0)

            io_b = io16_t[:].rearrange("p (u v) -> p u v", u=1).to_broadcast(
                [P, tpb, P])
            qcall = [0]

            # ---- phase 0: slice1 rows = [x@W1 | 1 | as1 | ad1] ----
            for b in range(nblk):
                t1T_ps = ps.tile([TW, P], f32, tag="pst")
                nc.tensor.matmul(t1T_ps[:], lhsT=w1_t[:],
                                 rhs=xT_t[:, b * P:(b + 1) * P],
                                 start=True, stop=True)
                t1T_sb = sb.tile([TW, P], f16, tag="t1Tsb")
                nc.vector.tensor_copy(t1T_sb[:], t1T_ps[:])
                t1_ps = ps.tile([P, TW], f16, tag="pst")
                nc.tensor.transpose(t1_ps[:], t1T_sb[:], id16_t[:TW, :TW])
                t1_sb = sb.tile([P, TW], f16, tag="t1sb")
                nc.vector.tensor_copy(t1_sb[:], t1_ps[:])
                nc.vector.memset(t1_sb[:, ONEC:ONEC + 1], 1.0)
                nc.sync.dma_start(slice1[b * P:(b + 1) * P, 0:TW], t1_sb[:])

            nc.gpsimd.collective_compute(
                "AllGather", Alu.bypass,
                replica_groups=[list(range(ncores))],
                ins=[slice1.opt()], outs=[table1[:, :]],
            )

            def gat_layer(table_h, slice_ap, is_last):
                lname = "L2" if is_last else "L1"
                # ad[dst] for local nodes: ad_grid[p, b] = slice[b*128+p, ADC]
                ad_grid = cst.tile([P, nblk], f16, name=f"adg{lname}")
                nc.sync.dma_start(
                    ad_grid[:],
                    slice_ap[:, ADC:ADC + 1].rearrange(
                        "(b p) c -> p (b c)", p=P),
                )
                for ch in range(nchunk):
                    gq3 = []
                    for q in range(NQ):
                        idxq = sb.tile([P, chunk * idxw], i16, tag=f"idx{q}")
                        nc.sync.dma_start(
                            idxq[:],
                            idx_d[:, (q * nblk + ch * chunk) * idxw:
                                  (q * nblk + (ch + 1) * chunk) * idxw])
                        g = gat.tile([P, chunk * tpq * ROW], f16,
                                     tag=f"g{q}")
                        g3 = g[:].rearrange("p (c e) -> p c e", e=ROW)
                        for j in range(chunk):
                            nc.gpsimd.dma_gather(
                                out_ap=g3[:, j * tpq:(j + 1) * tpq, :],
                                in_ap=table_h[q * qrows:(q + 1) * qrows, :],
                                idxs_ap=idxq[:, j * idxw:(j + 1) * idxw],
                                num_idxs=nidx,
                                num_idxs_reg=nidx,
                                elem_size=ROW,
                                queue_num=qcall[0] % 4,
                            )
                            qcall[0] += 1
                        gq3.append(g3)
                    # self-loop rows of this chunk's blocks (local, seq DMA)
                    sf = gat.tile([P, chunk * ROW], f16, tag="self")
                    nc.sync.dma_start(
                        sf[:].rearrange("p (b e) -> p b e", e=ROW),
                        slice_ap[ch * chunk * P:(ch + 1) * chunk * P,
                                 :].rearrange("(b p) e -> p b e", p=P))

                    for j in range(chunk):
                        b = ch * chunk + j
                        # as[src_e] per edge slot (col ASC of gathered rows)
                        as_all = sb.tile([P, tpb], f16, tag="asall")
                        for q in range(NQ):
                            nc.vector.tensor_copy(
                                as_all[:, q * tpq:(q + 1) * tpq].rearrange(
                                    "p (t u) -> p t u", u=1),
                                gq3[q][:, j * tpq:(j + 1) * tpq,
                                       ASC:ASC + 1])
                        # adR[p, v] = ad of dst v in this block
                        ad_blk = sb.tile([P, 1], f16, tag="adblk")
                        nc.vector.tensor_copy(ad_blk[:],
                                              ad_grid[:, b:b + 1])
                        adR_ps = ps.tile([P, P], f16, tag="pst")
                        nc.tensor.transpose(
                            adR_ps[:], ad_blk[:, 0:1].to_broadcast([P, P]),
                            id16_t[:])
                        adR = sb.tile([P, P], f16, tag="adR")
                        nc.vector.tensor_copy(adR[:], adR_ps[:])
                        # one-hot S over all tiles of the block
                        S_all = sb.tile([P, tpb * P], f16, tag="S")
                        nc.vector.tensor_tensor(
                            out=S_all[:].rearrange("p (t v) -> p t v", v=P),
                            in0=rel_t[:, b * tpb:(b + 1) * tpb].rearrange(
                                "p (t u) -> p t u", u=1).to_broadcast(
                                [P, tpb, P]),
                            in1=io_b, op=Alu.is_equal)
                        # X[p,t,v] = ad[v] + as[p,t]
                        X_all = sb.tile([P, tpb * P], f16, tag="X")
                        nc.vector.tensor_tensor(
                            out=X_all[:].rearrange("p (t v) -> p t v", v=P),
                            in0=adR[:].rearrange("p (u v) -> p u v",
                                                 u=1).to_broadcast(
                                [P, tpb, P]),
                            in1=as_all[:].rearrange("p (t u) -> p t u",
                                                    u=1).to_broadcast(
                                [P, tpb, P]),
                            op=Alu.add)
                        # exp(leakyrelu(x)) = max(exp(x), exp(0.2 x));
                        # both Exp -> no activation-table thrash
                        E2 = sb.tile([P, tpb * P], f16, tag="E2")
                        nc.scalar.activation(E2[:], X_all[:], Act.Exp,
                                             scale=0.2)
                        nc.scalar.activation(X_all[:], X_all[:], Act.Exp)
                        nc.vector.tensor_tensor(out=X_all[:], in0=X_all[:],
                                                in1=E2[:], op=Alu.max)
                        nc.vector.tensor_tensor(out=S_all[:], in0=S_all[:],
                                                in1=X_all[:], op=Alu.mult)
                        # self loop weight from the local row
                        xes = sb.tile([P, 1], f32, tag="xes")
                        nc.vector.tensor_tensor(
                            out=xes[:], in0=sf[:, j * ROW + ASC:
                                               j * ROW + ASC + 1],
                            in1=sf[:, j * ROW + ADC:j * ROW + ADC + 1],
                            op=Alu.add)
                        e2s = sb.tile([P, 1], f32, tag="e2s")
                        nc.scalar.activation(e2s[:], xes[:], Act.Exp,
                                             scale=0.2)
                        nc.scalar.activation(xes[:], xes[:], Act.Exp)
                        nc.vector.tensor_tensor(out=xes[:], in0=xes[:],
                                                in1=e2s[:], op=Alu.max)
                        exSs = sb.tile([P, P], f16, tag="exSs")
                        nc.vector.tensor_tensor(
                            out=exSs[:], in0=id16_t[:],
                            in1=xes[:].to_broadcast([P, P]), op=Alu.mult)

                        acc = psacc.tile([P, RW], f32, tag="acc")
                        for q in range(NQ):
                            for i in range(tpq):
                                t = q * tpq + i
                                nc.tensor.matmul(
                                    acc[:],
                                    lhsT=S_all[:, t * P:(t + 1) * P],
                                    rhs=gq3[q][:, j * tpq + i:
                                               j * tpq + i + 1,
                                               0:RW].rearrange(
                                        "p c e -> p (c e)"),
                                    start=(t == 0), stop=False)
                        nc.tensor.matmul(
                            acc[:], lhsT=exSs[:],
                            rhs=sf[:, j * ROW:j * ROW + RW],
                            start=False, stop=True)

                        den = sb.tile([P, 1], f32, tag="den")
                        nc.vector.tensor_scalar(
                            out=den[:], in0=acc[:, HID:HID + 1],
                            scalar1=1e-30, scalar2=None, op0=Alu.max)
                        rec = sb.tile([P, 1], f32, tag="rec")
                        nc.vector.reciprocal(rec[:], den[:])
                        hv = sb.tile([P, HID], f32, tag="hv")
                        nc.vector.tensor_tensor(
                            out=hv[:], in0=acc[:, 0:HID],
                            in1=rec[:].to_broadcast([P, HID]), op=Alu.mult)
                        if not is_last:
                            nc.vector.tensor_tensor(out=hv[:], in0=hv[:],
                                                    in1=b1_t[:], op=Alu.add)
                            nc.vector.tensor_scalar(out=hv[:], in0=hv[:],
                                                    scalar1=0.0, scalar2=None,
                                                    op0=Alu.max)
                            hv16 = sb.tile([P, HID], f16, tag="hv16")
                            nc.vector.tensor_copy(hv16[:], hv[:])
                            hvT_ps = ps.tile([HID, P], f16, tag="pst")
                            nc.tensor.transpose(hvT_ps[:], hv16[:], id16_t[:])
                            hvT = sb.tile([HID, P], f16, tag="hvT")
                            nc.vector.tensor_copy(hvT[:], hvT_ps[:])
                            t2T_ps = ps.tile([TW, P], f32, tag="pst")
                            nc.tensor.matmul(t2T_ps[:], lhsT=w2_t[:],
                                             rhs=hvT[:], start=True,
                                             stop=True)
                            t2T_sb = sb.tile([TW, P], f16, tag="t2Tsb")
                            nc.vector.tensor_copy(t2T_sb[:], t2T_ps[:])
                            t2_ps = ps.tile([P, TW], f16, tag="pst")
                            nc.tensor.transpose(t2_ps[:], t2T_sb[:],
                                                id16_t[:TW, :TW])
                            t2_sb = sb.tile([P, TW], f16, tag="t2sb")
                            nc.vector.tensor_copy(t2_sb[:], t2_ps[:])
                            nc.vector.memset(t2_sb[:, ONEC:ONEC + 1], 1.0)
                            nc.sync.dma_start(
                                slice2[b * P:(b + 1) * P, 0:TW], t2_sb[:])
                        else:
                            nc.vector.tensor_tensor(out=hv[:], in0=hv[:],
                                                    in1=b2_t[:], op=Alu.add)
                            prhs = sb.tile([P, HID + 1], f16, tag="prhs")
                            nc.vector.tensor_copy(prhs[:, 0:HID], hv[:])
                            nc.vector.tensor_copy(prhs[:, HID:HID + 1],
                                                  one16_t[:])
                            gid_col = sb.tile([P, 1], f16, tag="gidcol")
                            nc.vector.tensor_copy(gid_col[:],
                                                  gid_t[:, b:b + 1])
                            Gh = sb.tile([P, P], f16, tag="Gh")
                            nc.vector.tensor_tensor(
                                out=Gh[:],
                                in0=gid_col[:].to_broadcast([P, P]),
                                in1=io16_t[:], op=Alu.is_equal)
                            pool_ps = ps.tile([P, HID + 1], f32, tag="pst")
                            nc.tensor.matmul(pool_ps[:], lhsT=Gh[:],
                                             rhs=prhs[:], start=True,
                                             stop=True)
                            nc.vector.tensor_tensor(out=pooled[:],
                                                    in0=pooled[:],
                                                    in1=pool_ps[:],
                                                    op=Alu.add)

            gat_layer(table1, slice1, is_last=False)
            nc.gpsimd.collective_compute(
                "AllGather", Alu.bypass,
                replica_groups=[list(range(ncores))],
                ins=[slice2.opt()], outs=[table2[:, :]],
            )
            gat_layer(table2, slice2, is_last=True)

            # ---- AllReduce pooled sums ----
            nc.sync.dma_start(pool_in[:, :], pooled[:])
            nc.gpsimd.collective_compute(
                "AllReduce", Alu.add,
                replica_groups=[list(range(ncores))],
                ins=[pool_in.opt()], outs=[pool_out.opt()],
            )
            pl = sb.tile([P, HID + 1], f32, tag="pl")
            nc.sync.dma_start(pl[:], pool_out[:, :])

            cnt = sb.tile([P, 1], f32, tag="cnt")
            nc.vector.tensor_scalar(out=cnt[:], in0=pl[:, HID:HID + 1],
                                    scalar1=1.0, scalar2=None, op0=Alu.max)
            crec = sb.tile([P, 1], f32, tag="crec")
            nc.vector.reciprocal(crec[:], cnt[:])
            mean = sb.tile([P, HID], f32, tag="mean")
            nc.vector.tensor_tensor(out=mean[:], in0=pl[:, 0:HID],
                                    in1=crec[:].to_broadcast([P, HID]),
                                    op=Alu.mult)

            # MLP: z = relu(mean @ lin_w + lin_b); logits = z @ cls_w + cls_b
            lw_t = cst.tile([HID, HID // 2], f32)
            nc.sync.dma_start(lw_t[:], lw_d[:, :])
            lb_t = cst.tile([HID // 2, 1], f32)
            nc.sync.dma_start(lb_t[:], lb_d[:, :])
            cw_t = cst.tile([HID // 2, C], f32)
            nc.sync.dma_start(cw_t[:], cw_d[:, :])
            cb_t = cst.tile([C, 1], f32)
            nc.sync.dma_start(cb_t[:], cb_d[:, :])

            meanT_ps = ps.tile([HID, P], f32, tag="pst")
            nc.tensor.transpose(meanT_ps[:], mean[:], id32_t[:])
            meanT = sb.tile([HID, P], f32, tag="meanT")
            nc.vector.tensor_copy(meanT[:], meanT_ps[:])
            zT_ps = ps.tile([HID // 2, P], f32, tag="pst")
            nc.tensor.matmul(zT_ps[:], lhsT=lw_t[:], rhs=meanT[:],
                             start=True, stop=True)
            zT = sb.tile([HID // 2, P], f32, tag="zT")
            nc.scalar.activation(zT[:], zT_ps[:], Act.Relu, bias=lb_t[:])
            lgT_ps = ps.tile([C, P], f32, tag="pst")
            nc.tensor.matmul(lgT_ps[:], lhsT=cw_t[:], rhs=zT[:],
                             start=True, stop=True)
            lgT = sb.tile([C, P], f32, tag="lgT")
            nc.scalar.activation(lgT[:], lgT_ps[:], Act.Identity, bias=cb_t[:])
            lg_ps = ps.tile([P, C], f32, tag="pst")
            nc.tensor.transpose(lg_ps[:], lgT[:], id32_t[:C, :C])
            lg = sb.tile([P, C], f32, tag="lg")
            nc.vector.tensor_copy(lg[:], lg_ps[:])

            mx = sb.tile([P, 1], f32, tag="mx")
            nc.vector.tensor_reduce(mx[:], lg[:], axis=mybir.AxisListType.X,
                                    op=Alu.max)
            sh = sb.tile([P, C], f32, tag="sh")
            nc.vector.tensor_tensor(out=sh[:], in0=lg[:],
                                    in1=mx[:].to_broadcast([P, C]),
                                    op=Alu.subtract)
            exs = sb.tile([P, C], f32, tag="exs")
            se = sb.tile([P, 1], f32, tag="se")
            nc.scalar.activation(exs[:], sh[:], Act.Exp, accum_out=se[:])
            lse = sb.tile([P, 1], f32, tag="lse")
            nc.scalar.activation(lse[:], se[:], Act.Ln)
            res = sb.tile([P, C], f32, tag="res")
            nc.vector.tensor_tensor(out=res[:], in0=sh[:],
                                    in1=lse[:].to_broadcast([P, C]),
                                    op=Alu.subtract)
            nc.sync.dma_start(out_d[:, :], res[:])

    nc.compile()
    return nc


def run_gnn(inputs, ncores=8, trace=False):
    from concourse.bass_utils import run_bass_kernel_spmd

    x = np.asarray(inputs["x"], np.float32)
    edge_index = np.asarray(inputs["edge_index"])
    batch = np.asarray(inputs["batch"])
    W1 = np.asarray(inputs["W1"], np.float32)
    W2 = np.asarray(inputs["W2"], np.float32)
    hd = _build_host_data(
        x, edge_index, batch, W1,
        np.asarray(inputs["a_src1"], np.float32),
        np.asarray(inputs["a_dst1"], np.float32),
        W2,
        np.asarray(inputs["a_src2"], np.float32),
        np.asarray(inputs["a_dst2"], np.float32),
        ncores)

    N, F_IN = x.shape
    G = 128  # number of graphs == P (pooling one-hot relies on this)
    C = np.asarray(inputs["cls_w"]).shape[1]

    nc = _build_program(ncores, hd["nblk"], hd["tpq"], F_IN, G, C,
                        hd["V"], hd["qrows"], hd["nchunk"], hd["chunk"])

    iota16 = np.tile(np.arange(P, dtype=np.float16)[None, :], (P, 1))
    ident16 = np.eye(P, dtype=np.float16)
    ident32 = np.eye(P, dtype=np.float32)
    ones16 = np.ones((P, 1), np.float16)
    b1rep = np.tile(np.asarray(inputs["b1"], np.float32)[None, :], (P, 1))
    b2rep = np.tile(np.asarray(inputs["b2"], np.float32)[None, :], (P, 1))

    in_maps = []
    for c in range(ncores):
        in_maps.append({
            "xT": hd["xT_all"][c],
            "idx": hd["idx_all"][c],
            "rel": hd["rel_all"][c],
            "gid": hd["gid_all"][c],
            "w1aug": hd["W1aug"],
            "w2aug": hd["W2aug"],
            "b1rep": b1rep,
            "b2rep": b2rep,
            "lin_w": np.asarray(inputs["lin_w"], np.float32),
            "lin_b": np.asarray(inputs["lin_b"], np.float32)[:, None],
            "cls_w": np.asarray(inputs["cls_w"], np.float32),
            "cls_b": np.asarray(inputs["cls_b"], np.float32)[:, None],
            "iota16": iota16,
            "ident16": ident16,
            "ident32": ident32,
            "ones16": ones16,
        })

    res = run_bass_kernel_spmd(nc, in_maps, core_ids=list(range(ncores)),
                               trace=trace)
    out = res.results[0]["out"]
    return out, res


def kernel(**inputs):
    out, _ = run_gnn(inputs, ncores=8)
    return out.astype(np.float32)


# revision 22
# speedup vs baseline: 2.3673x; 1.2039x over previous
"""GAT (2-layer) + global mean pool + MLP + log_softmax on 8 Trainium2 cores.

Strategy (dst-sharded message passing, bulk-gather edition):
  - Nodes partitioned across 8 cores; per-core node tables
    ([h@W | 1 | as | ad] as 128-col f16 rows = 256B) are computed shard-wise
    and replicated via AllGather into DRAM tables.
  - Per-edge h[src] rows are fetched with Pool-engine dma_gather (<=1024
    indices per call - the ucode limit), one call per (dst-block, quarter),
    rotated over 4 SWDGE queues (descriptor generation parallelizes ~3.6x
    across queues).  The table is addressed in 4 quarters so indices fit
    int16.
  - Local nodes are bin-packed into blocks balancing per-(block, quarter)
    edge counts, so every bucket fits tpq tiles of 128 edges with minimal
    padding.  Self loops bypass the gather entirely (local rows, identity
    one-hot).
  - Per-edge attention weights use exp(leakyrelu(x)) = max(exp(x),
    exp(0.2x)): two Exp passes on the otherwise-idle scalar engine (table
    pinned to Exp), so the vector engine only builds the one-hot, one add
    and one max per block.
  - Aggregation is a PSUM-accumulated one-hot matmul per 128-edge tile with
    the softmax denominator riding along as the table's constant-1 column.
  - Graph pooling = one-hot matmul + AllReduce; tiny MLP + log_softmax run
    redundantly on every core.
"""

import sys

sys.path.insert(0, "/opt/trn_rl_repo")

import numpy as np

P = 128
NQ = 4          # src-quarters (int16 index range per dma_gather)
ROW = 128       # table row width in f16 elems (256B, dma_gather granularity)
HID = 64
ONEC = HID      # constant-1 column (denominator rides the matmul)
ASC = HID + 1   # alpha_src column
ADC = HID + 2   # alpha_dst column
TW = HID + 3    # populated row prefix
RW = HID + 1    # rhs width for the aggregation matmul: [h | 1]
CHUNK = 10      # dst blocks per gather chunk


def _pack_blocks(deg_q, nblk, cap):
    """First-fit-decreasing bin packing: assign nodes (rows of deg_q
    [npc, NQ]) to nblk bins with <= P nodes per bin and per-quarter edge
    count <= cap.  Returns pos[npc] (slot b*P + i) or None."""
    npc = deg_q.shape[0]
    order = np.argsort(-deg_q.sum(1), kind="stable")
    rem = np.full((nblk, NQ), cap, np.int64)
    cnt = np.zeros(nblk, np.int64)
    pos = np.empty(npc, np.int64)
    for l in order:
        ok = (cnt < P) & (rem >= deg_q[l]).all(1)
        b = int(np.argmax(ok))
        if not ok[b]:
            return None
        pos[l] = b * P + cnt[b]
        cnt[b] += 1
        rem[b] -= deg_q[l]
    return pos


def _build_host_data(x, edge_index, batch, W1, a_src1, a_dst1, W2, a_src2,
                     a_dst2, ncores):
    """Pure-integer/graph preprocessing + augmented weights (host side)."""
    N, F_IN = x.shape
    assert N % ncores == 0 and ncores % NQ == 0
    npc = N // ncores
    nblk = -(-npc // P)
    chunk = min(CHUNK, nblk)
    nblk = -(-nblk // chunk) * chunk
    npc_pad = nblk * P
    V = ncores * npc_pad
    qrows = V // NQ
    assert qrows <= 32767
    nchunk = nblk // chunk

    # self loops are handled separately on-device (local rows, no gather)
    src = np.asarray(edge_index[0])
    dst = np.asarray(edge_index[1])
    score = (src // npc).astype(np.int64)
    dcore = (dst // npc).astype(np.int64)
    sloc = (src % npc).astype(np.int64)
    dloc = (dst % npc).astype(np.int64)

    # per-node quarter in-degree, then degree-balanced packing into blocks
    deg = np.zeros((ncores, npc, NQ), np.int64)
    # quarter of the src depends on its packed position; quarters span whole
    # cores (qrows is a multiple of npc_pad * cores-per-quarter), so the
    # quarter is known before packing:
    cpq = ncores // NQ
    q_of = score // cpq
    np.add.at(deg, (dcore, dloc, q_of), 1)

    pos_all = np.empty((ncores, npc), np.int64)
    tpq = None
    for cap_tiles in range(4, 9):
        ok = True
        for c in range(ncores):
            pos = _pack_blocks(deg[c], nblk, cap_tiles * P)
            if pos is None:
                ok = False
                break
            pos_all[c] = pos
        if ok:
            tpq = cap_tiles
            break
    assert tpq is not None
    assert tpq * P <= 1024  # dma_gather ucode faults above 1024 indices
    tpb = NQ * tpq

    srow = score * npc_pad + pos_all[score, sloc]
    qoff = (srow % qrows).astype(np.int64)
    dpos = pos_all[dcore, dloc]
    blk = dpos // P
    rel = dpos % P

    cnt = np.zeros((ncores, nblk, NQ), np.int64)
    np.add.at(cnt, (dcore, blk, q_of), 1)
    tpq = int(max(1, -(-cnt.max() // P)))
    tpb = NQ * tpq

    call_idx = np.zeros((ncores, NQ, nblk, tpq * P), np.int16)
    rel_all = np.full((ncores, P, nblk * tpb), -1.0, np.float16)

    order = np.lexsort((q_of, blk, dcore))
    so_q, so_b, so_c = q_of[order], blk[order], dcore[order]
    so_qoff, so_rel = qoff[order], rel[order]
    key = (so_c * nblk + so_b) * NQ + so_q
    start = np.searchsorted(key, np.arange(ncores * nblk * NQ), side="left")
    pos = np.arange(len(key)) - start[key]
    call_idx[so_c, so_q, so_b, pos] = so_qoff.astype(np.int16)
    rel_col = so_b * tpb + so_q * tpq + pos // P
    rel_all[so_c, pos % P, rel_col] = so_rel.astype(np.float16)

    # wrap indices for the gpsimd cores: idx j lives at [p % 16 == j % 16,
    # j // 16], replicated across the 8 groups of 16 partitions
    ci = call_idx.reshape(ncores, NQ, nblk, -1, 16)
    ci = np.transpose(ci, (0, 1, 2, 4, 3))
    ci = np.broadcast_to(ci[:, :, :, None, :, :],
                         (ncores, NQ, nblk, 8, 16, tpq * P // 16))
    idx_all = ci.reshape(ncores, NQ, nblk, P, -1)
    idx_all = np.transpose(idx_all, (0, 3, 1, 2, 4)).reshape(ncores, P, -1)
    idx_all = np.ascontiguousarray(idx_all)

    gid_all = np.full((ncores, P, nblk), -1.0, np.float32)
    xT_all = np.zeros((ncores, F_IN, npc_pad), np.float16)
    for c in range(ncores):
        ids = np.arange(npc)
        gg = np.full(npc_pad, -1.0, np.float32)
        gg[pos_all[c]] = batch[ids + c * npc].astype(np.float32)
        gid_all[c] = gg.reshape(nblk, P).T
        xT_all[c][:, pos_all[c]] = x[c * npc:(c + 1) * npc].T.astype(
            np.float16)

    def aug(W, a_s, a_d):
        w = np.zeros((W.shape[0], TW), np.float32)
        w[:, :HID] = W
        w[:, ASC] = W @ a_s
        w[:, ADC] = W @ a_d
        return w.astype(np.float16)

    return dict(npc=npc, nblk=nblk, npc_pad=npc_pad, tpq=tpq, tpb=tpb, V=V,
                chunk=chunk, qrows=qrows, nchunk=nchunk, idx_all=idx_all,
                rel_all=rel_all, gid_all=gid_all, xT_all=xT_all,
                W1aug=aug(W1, a_src1, a_dst1), W2aug=aug(W2, a_src2, a_dst2))


def _build_program(ncores, nblk, tpq, F_IN, G, C, V, qrows, nchunk, chunk):
    import concourse.bass as bass
    import concourse.bacc as bacc
    import concourse.tile as tile
    from concourse import mybir

    tpb = NQ * tpq
    npc_pad = nblk * P
    nidx = tpq * P                    # indices per (block, quarter) gather
    idxw = nidx // 16                 # idx cols per call (int16, wrapped)

    nc = bacc.Bacc("TRN2", target_bir_lowering=False, debug=False,
                   num_devices=ncores, num_swdge_queues=4)
    f32, f16, i16 = mybir.dt.float32, mybir.dt.float16, mybir.dt.int16
    Alu = mybir.AluOpType
    Act = mybir.ActivationFunctionType

    ein = lambda n, s, d: nc.dram_tensor(n, s, d, kind="ExternalInput")
    xT_d = ein("xT", [F_IN, npc_pad], f16)
    idx_d = ein("idx", [P, NQ * nblk * idxw], i16)
    rel_d = ein("rel", [P, nblk * tpb], f16)
    gid_d = ein("gid", [P, nblk], f32)
    w1_d = ein("w1aug", [F_IN, TW], f16)
    w2_d = ein("w2aug", [HID, TW], f16)
    b1_d = ein("b1rep", [P, HID], f32)
    b2_d = ein("b2rep", [P, HID], f32)
    lw_d = ein("lin_w", [HID, HID // 2], f32)
    lb_d = ein("lin_b", [HID // 2, 1], f32)
    cw_d = ein("cls_w", [HID // 2, C], f32)
    cb_d = ein("cls_b", [C, 1], f32)
    io16_d = ein("iota16", [P, P], f16)
    id16_d = ein("ident16", [P, P], f16)
    id32_d = ein("ident32", [P, P], f32)
    one16_d = ein("ones16", [P, 1], f16)
    out_d = nc.dram_tensor("out", [G, C], f32, kind="ExternalOutput")

    table1 = nc.dram_tensor("table1", [V, ROW], f16, kind="Internal")
    table2 = nc.dram_tensor("table2", [V, ROW], f16, kind="Internal")

    with tile.TileContext(nc) as tc:
        with (
            tc.tile_pool(name="cst", bufs=1) as cst,
            tc.tile_pool(name="sb", bufs=3) as sb,
            tc.tile_pool(name="gat", bufs=2) as gat,
            tc.tile_pool(name="ps", bufs=4, space="PSUM") as ps,
            tc.tile_pool(name="psacc", bufs=3, space="PSUM") as psacc,
            tc.tile_pool(name="pspool", bufs=1, space="PSUM") as pspool,
            tc.tile_pool(name="dram", bufs=1, space="DRAM") as dram,
        ):
            # ---- constants ----
            rel_t = cst.tile([P, nblk * tpb], f16)
            nc.sync.dma_start(rel_t[:], rel_d[:, :])
            gid_t = cst.tile([P, nblk], f32)
            nc.sync.dma_start(gid_t[:], gid_d[:, :])
            w1_t = cst.tile([F_IN, TW], f16)
            nc.sync.dma_start(w1_t[:], w1_d[:, :])
            w2_t = cst.tile([HID, TW], f16)
            nc.sync.dma_start(w2_t[:], w2_d[:, :])
            b1_t = cst.tile([P, HID], f32)
            nc.sync.dma_start(b1_t[:], b1_d[:, :])
            b2_t = cst.tile([P, HID], f32)
            nc.sync.dma_start(b2_t[:], b2_d[:, :])
            io16_t = cst.tile([P, P], f16)
            nc.sync.dma_start(io16_t[:], io16_d[:, :])
            id16_t = cst.tile([P, P], f16)
            nc.sync.dma_start(id16_t[:], id16_d[:, :])
            id32_t = cst.tile([P, P], f32)
            nc.sync.dma_start(id32_t[:], id32_d[:, :])
            one16_t = cst.tile([P, 1], f16)
            nc.sync.dma_start(one16_t[:], one16_d[:, :])
            xT_t = cst.tile([F_IN, npc_pad], f16)
            nc.sync.dma_start(xT_t[:], xT_d[:, :])

            slice1 = dram.tile([npc_pad, ROW], f16)
            slice2 = dram.tile([npc_pad, ROW], f16)
            pool_in = dram.tile([P, HID + 1], f32)
            pool_out = dram.tile([P, HID + 1], f32)

            pooled_ps = pspool.tile([P, HID + 1], f32)

            io_b = io16_t[:].rearrange("p (u v) -> p u v", u=1).to_broadcast(
                [P, tpb, P])
            qcall = [0]

            # ---- phase 0: slice1 rows = [x@W1 | 1 | as1 | ad1] ----
            for b in range(nblk):
                t1T_ps = ps.tile([TW, P], f32, tag="pst")
                nc.tensor.matmul(t1T_ps[:], lhsT=w1_t[:],
                                 rhs=xT_t[:, b * P:(b + 1) * P],
                                 start=True, stop=True)
                t1T_sb = sb.tile([TW, P], f16, tag="t1Tsb")
                nc.vector.tensor_copy(t1T_sb[:], t1T_ps[:])
                t1_ps = ps.tile([P, TW], f16, tag="pst")
                nc.tensor.transpose(t1_ps[:], t1T_sb[:], id16_t[:TW, :TW])
                t1_sb = sb.tile([P, TW], f16, tag="t1sb")
                nc.vector.tensor_copy(t1_sb[:], t1_ps[:])
                nc.vector.memset(t1_sb[:, ONEC:ONEC + 1], 1.0)
                nc.sync.dma_start(slice1[b * P:(b + 1) * P, 0:TW], t1_sb[:])

            nc.gpsimd.collective_compute(
                "AllGather", Alu.bypass,
                replica_groups=[list(range(ncores))],
                ins=[slice1.opt()], outs=[table1[:, :]],
            )

            def gat_layer(table_h, slice_ap, is_last):
                lname = "L2" if is_last else "L1"
                # ad[dst] for local nodes: ad_grid[p, b] = slice[b*128+p, ADC]
                ad_grid = cst.tile([P, nblk], f16, name=f"adg{lname}")
                nc.sync.dma_start(
                    ad_grid[:],
                    slice_ap[:, ADC:ADC + 1].rearrange(
                        "(b p) c -> p (b c)", p=P),
                )
                for ch in range(nchunk):
                    gq3 = []
                    for q in range(NQ):
                        idxq = sb.tile([P, chunk * idxw], i16, tag=f"idx{q}")
                        nc.sync.dma_start(
                            idxq[:],
                            idx_d[:, (q * nblk + ch * chunk) * idxw:
                                  (q * nblk + (ch + 1) * chunk) * idxw])
                        g = gat.tile([P, chunk * tpq * ROW], f16,
                                     tag=f"g{q}")
                        g3 = g[:].rearrange("p (c e) -> p c e", e=ROW)
                        # pack as many blocks per call as the 1024-index
                        # ucode limit allows
                        bpc = max(1, 1024 // nidx)
                        while chunk % bpc:
                            bpc -= 1
                        for j in range(0, chunk, bpc):
                            nc.gpsimd.dma_gather(
                                out_ap=g3[:, j * tpq:(j + bpc) * tpq, :],
                                in_ap=table_h[q * qrows:(q + 1) * qrows, :],
                                idxs_ap=idxq[:, j * idxw:(j + bpc) * idxw],
                                num_idxs=nidx * bpc,
                                num_idxs_reg=nidx * bpc,
                                elem_size=ROW,
                                queue_num=qcall[0] % 4,
                            )
                            qcall[0] += 1
                        gq3.append(g3)
                    # self-loop rows of this chunk's blocks (local, seq DMA)
                    sf = gat.tile([P, chunk * ROW], f16, tag="self")
                    nc.sync.dma_start(
                        sf[:].rearrange("p (b e) -> p b e", e=ROW),
                        slice_ap[ch * chunk * P:(ch + 1) * chunk * P,
                                 :].rearrange("(b p) e -> p b e", p=P))

                    for j in range(chunk):
                        b = ch * chunk + j
                        # adR[p, v] = ad of dst v in this block
                        ad_blk = sb.tile([P, 1], f16, tag="adblk")
                        nc.vector.tensor_copy(ad_blk[:],
                                              ad_grid[:, b:b + 1])
                        adR_ps = ps.tile([P, P], f16, tag="pst")
                        nc.tensor.transpose(
                            adR_ps[:], ad_blk[:, 0:1].to_broadcast([P, P]),
                            id16_t[:])
                        adR = sb.tile([P, P], f16, tag="adR")
                        nc.vector.tensor_copy(adR[:], adR_ps[:])
                        # one-hot S over all tiles of the block
                        S_all = sb.tile([P, tpb * P], f16, tag="S")
                        nc.vector.tensor_tensor(
                            out=S_all[:].rearrange("p (t v) -> p t v", v=P),
                            in0=rel_t[:, b * tpb:(b + 1) * tpb].rearrange(
                                "p (t u) -> p t u", u=1).to_broadcast(
                                [P, tpb, P]),
                            in1=io_b, op=Alu.is_equal)
                        # X[p,t,v] = ad[v] + as[p,t], as read straight from
                        # the gathered rows (col ASC), one op per quarter
                        X_all = sb.tile([P, tpb * P], f16, tag="X")
                        adR_b1 = adR[:].rearrange(
                            "p (u v) -> p u v", u=1).to_broadcast(
                            [P, tpq, P])
                        for q in range(NQ):
                            nc.vector.tensor_tensor(
                                out=X_all[:, q * tpq * P:
                                          (q + 1) * tpq * P].rearrange(
                                    "p (t v) -> p t v", v=P),
                                in0=adR_b1,
                                in1=gq3[q][:, j * tpq:(j + 1) * tpq,
                                           ASC:ASC + 1].to_broadcast(
                                    [P, tpq, P]),
                                op=Alu.add)
                        # exp(leakyrelu(x)) = max(exp(x), exp(0.2 x));
                        # both Exp -> no activation-table thrash
                        E2 = sb.tile([P, tpb * P], f16, tag="E2")
                        nc.scalar.activation(E2[:], X_all[:], Act.Exp,
                                             scale=0.2)
                        nc.scalar.activation(X_all[:], X_all[:], Act.Exp)
                        nc.vector.tensor_tensor(out=X_all[:], in0=X_all[:],
                                                in1=E2[:], op=Alu.max)
                        nc.vector.tensor_tensor(out=S_all[:], in0=S_all[:],
                                                in1=X_all[:], op=Alu.mult)
                        # self loop weight from the local row
                        xes = sb.tile([P, 1], f32, tag="xes")
                        nc.vector.tensor_tensor(
                            out=xes[:], in0=sf[:, j * ROW + ASC:
                                               j * ROW + ASC + 1],
                            in1=sf[:, j * ROW + ADC:j * ROW + ADC + 1],
                            op=Alu.add)
                        e2s = sb.tile([P, 1], f32, tag="e2s")
                        nc.scalar.activation(e2s[:], xes[:], Act.Exp,
                                             scale=0.2)
                        nc.scalar.activation(xes[:], xes[:], Act.Exp)
                        nc.vector.tensor_tensor(out=xes[:], in0=xes[:],
                                                in1=e2s[:], op=Alu.max)
                        exSs = sb.tile([P, P], f16, tag="exSs")
                        nc.vector.tensor_tensor(
                            out=exSs[:], in0=id16_t[:],
                            in1=xes[:].to_broadcast([P, P]), op=Alu.mult)

                        acc = psacc.tile([P, RW], f32, tag="acc")
                        for q in range(NQ):
                            for i in range(tpq):
                                t = q * tpq + i
                                nc.tensor.matmul(
                                    acc[:],
                                    lhsT=S_all[:, t * P:(t + 1) * P],
                                    rhs=gq3[q][:, j * tpq + i:
                                               j * tpq + i + 1,
                                               0:RW].rearrange(
                                        "p c e -> p (c e)"),
                                    start=(t == 0), stop=False)
                        nc.tensor.matmul(
                            acc[:], lhsT=exSs[:],
                            rhs=sf[:, j * ROW:j * ROW + RW],
                            start=False, stop=True)

                        den = sb.tile([P, 1], f32, tag="den")
                        nc.vector.tensor_scalar(
                            out=den[:], in0=acc[:, HID:HID + 1],
                            scalar1=1e-30, scalar2=None, op0=Alu.max)
                        rec = sb.tile([P, 1], f32, tag="rec")
                        nc.vector.reciprocal(rec[:], den[:])
                        hv = sb.tile([P, HID], f32, tag="hv")
                        nc.vector.tensor_tensor(
                            out=hv[:], in0=acc[:, 0:HID],
                            in1=rec[:].to_broadcast([P, HID]), op=Alu.mult)
                        if not is_last:
                            nc.vector.tensor_tensor(out=hv[:], in0=hv[:],
                                                    in1=b1_t[:], op=Alu.add)
                            nc.vector.tensor_scalar(out=hv[:], in0=hv[:],
                                                    scalar1=0.0, scalar2=None,
                                                    op0=Alu.max)
                            hv16 = sb.tile([P, HID], f16, tag="hv16")
                            nc.vector.tensor_copy(hv16[:], hv[:])
                            hvT_ps = ps.tile([HID, P], f16, tag="pst")
                            nc.tensor.transpose(hvT_ps[:], hv16[:], id16_t[:])
                            hvT = sb.tile([HID, P], f16, tag="hvT")
                            nc.vector.tensor_copy(hvT[:], hvT_ps[:])
                            t2T_ps = ps.tile([TW, P], f32, tag="pst")
                            nc.tensor.matmul(t2T_ps[:], lhsT=w2_t[:],
                                             rhs=hvT[:], start=True,
                                             stop=True)
                            t2T_sb = sb.tile([TW, P], f16, tag="t2Tsb")
                            nc.vector.tensor_copy(t2T_sb[:], t2T_ps[:])
                            t2_ps = ps.tile([P, TW], f16, tag="pst")
                            nc.tensor.transpose(t2_ps[:], t2T_sb[:],
                                                id16_t[:TW, :TW])
                            t2_sb = sb.tile([P, TW], f16, tag="t2sb")
                            nc.vector.tensor_copy(t2_sb[:], t2_ps[:])
                            nc.vector.memset(t2_sb[:, ONEC:ONEC + 1], 1.0)
                            nc.sync.dma_start(
                                slice2[b * P:(b + 1) * P, 0:TW], t2_sb[:])
                        else:
                            nc.vector.tensor_tensor(out=hv[:], in0=hv[:],
                                                    in1=b2_t[:], op=Alu.add)
                            prhs = sb.tile([P, HID + 1], f16, tag="prhs")
                            nc.vector.tensor_copy(prhs[:, 0:HID], hv[:])
                            nc.vector.tensor_copy(prhs[:, HID:HID + 1],
                                                  one16_t[:])
                            gid_col = sb.tile([P, 1], f16, tag="gidcol")
                            nc.vector.tensor_copy(gid_col[:],
                                                  gid_t[:, b:b + 1])
                            Gh = sb.tile([P, P], f16, tag="Gh")
                            nc.vector.tensor_tensor(
                                out=Gh[:],
                                in0=gid_col[:].to_broadcast([P, P]),
                                in1=io16_t[:], op=Alu.is_equal)
                            nc.tensor.matmul(pooled_ps[:], lhsT=Gh[:],
                                             rhs=prhs[:], start=(b == 0),
                                             stop=(b == nblk - 1))

            gat_layer(table1, slice1, is_last=False)
            nc.gpsimd.collective_compute(
                "AllGather", Alu.bypass,
                replica_groups=[list(range(ncores))],
                ins=[slice2.opt()], outs=[table2[:, :]],
            )
            gat_layer(table2, slice2, is_last=True)

            # ---- AllReduce pooled sums ----
            pooled_sb = sb.tile([P, HID + 1], f32, tag="pooledsb")
            nc.vector.tensor_copy(pooled_sb[:], pooled_ps[:])
            nc.sync.dma_start(pool_in[:, :], pooled_sb[:])
            nc.gpsimd.collective_compute(
                "AllReduce", Alu.add,
                replica_groups=[list(range(ncores))],
                ins=[pool_in.opt()], outs=[pool_out.opt()],
            )
            pl = sb.tile([P, HID + 1], f32, tag="pl")
            nc.sync.dma_start(pl[:], pool_out[:, :])

            cnt = sb.tile([P, 1], f32, tag="cnt")
            nc.vector.tensor_scalar(out=cnt[:], in0=pl[:, HID:HID + 1],
                                    scalar1=1.0, scalar2=None, op0=Alu.max)
            crec = sb.tile([P, 1], f32, tag="crec")
            nc.vector.reciprocal(crec[:], cnt[:])
            mean = sb.tile([P, HID], f32, tag="mean")
            nc.vector.tensor_tensor(out=mean[:], in0=pl[:, 0:HID],
                                    in1=crec[:].to_broadcast([P, HID]),
                                    op=Alu.mult)

            # MLP: z = relu(mean @ lin_w + lin_b); logits = z @ cls_w + cls_b
            lw_t = cst.tile([HID, HID // 2], f32)
            nc.sync.dma_start(lw_t[:], lw_d[:, :])
            lb_t = cst.tile([HID // 2, 1], f32)
            nc.sync.dma_start(lb_t[:], lb_d[:, :])
            cw_t = cst.tile([HID // 2, C], f32)
            nc.sync.dma_start(cw_t[:], cw_d[:, :])
            cb_t = cst.tile([C, 1], f32)
            nc.sync.dma_start(cb_t[:], cb_d[:, :])

            meanT_ps = ps.tile([HID, P], f32, tag="pst")
            nc.tensor.transpose(meanT_ps[:], mean[:], id32_t[:])
            meanT = sb.tile([HID, P], f32, tag="meanT")
            nc.vector.tensor_copy(meanT[:], meanT_ps[:])
            zT_ps = ps.tile([HID // 2, P], f32, tag="pst")
            nc.tensor.matmul(zT_ps[:], lhsT=lw_t[:], rhs=meanT[:],
                             start=True, stop=True)
            zT = sb.tile([HID // 2, P], f32, tag="zT")
            nc.scalar.activation(zT[:], zT_ps[:], Act.Relu, bias=lb_t[:])
            lgT_ps = ps.tile([C, P], f32, tag="pst")
            nc.tensor.matmul(lgT_ps[:], lhsT=cw_t[:], rhs=zT[:],
                             start=True, stop=True)
            lgT = sb.tile([C, P], f32, tag="lgT")
            nc.scalar.activation(lgT[:], lgT_ps[:], Act.Identity, bias=cb_t[:])
            lg_ps = ps.tile([P, C], f32, tag="pst")
            nc.tensor.transpose(lg_ps[:], lgT[:], id32_t[:C, :C])
            lg = sb.tile([P, C], f32, tag="lg")
            nc.vector.tensor_copy(lg[:], lg_ps[:])

            mx = sb.tile([P, 1], f32, tag="mx")
            nc.vector.tensor_reduce(mx[:], lg[:], axis=mybir.AxisListType.X,
                                    op=Alu.max)
            sh = sb.tile([P, C], f32, tag="sh")
            nc.vector.tensor_tensor(out=sh[:], in0=lg[:],
                                    in1=mx[:].to_broadcast([P, C]),
                                    op=Alu.subtract)
            exs = sb.tile([P, C], f32, tag="exs")
            se = sb.tile([P, 1], f32, tag="se")
            nc.scalar.activation(exs[:], sh[:], Act.Exp, accum_out=se[:])
            lse = sb.tile([P, 1], f32, tag="lse")
            nc.scalar.activation(lse[:], se[:], Act.Ln)
            res = sb.tile([P, C], f32, tag="res")
            nc.vector.tensor_tensor(out=res[:], in0=sh[:],
                                    in1=lse[:].to_broadcast([P, C]),
                                    op=Alu.subtract)
            nc.sync.dma_start(out_d[:, :], res[:])

    nc.compile()
    return nc


def run_gnn(inputs, ncores=8, trace=False):
    from concourse.bass_utils import run_bass_kernel_spmd

    x = np.asarray(inputs["x"], np.float32)
    edge_index = np.asarray(inputs["edge_index"])
    batch = np.asarray(inputs["batch"])
    W1 = np.asarray(inputs["W1"], np.float32)
    W2 = np.asarray(inputs["W2"], np.float32)
    hd = _build_host_data(
        x, edge_index, batch, W1,
        np.asarray(inputs["a_src1"], np.float32),
        np.asarray(inputs["a_dst1"], np.float32),
        W2,
        np.asarray(inputs["a_src2"], np.float32),
        np.asarray(inputs["a_dst2"], np.float32),
        ncores)

    N, F_IN = x.shape
    G = 128  # number of graphs == P (pooling one-hot relies on this)
    C = np.asarray(inputs["cls_w"]).shape[1]

    nc = _build_program(ncores, hd["nblk"], hd["tpq"], F_IN, G, C,
                        hd["V"], hd["qrows"], hd["nchunk"], hd["chunk"])

    iota16 = np.tile(np.arange(P, dtype=np.float16)[None, :], (P, 1))
    ident16 = np.eye(P, dtype=np.float16)
    ident32 = np.eye(P, dtype=np.float32)
    ones16 = np.ones((P, 1), np.float16)
    b1rep = np.tile(np.asarray(inputs["b1"], np.float32)[None, :], (P, 1))
    b2rep = np.tile(np.asarray(inputs["b2"], np.float32)[None, :], (P, 1))

    in_maps = []
    for c in range(ncores):
        in_maps.append({
            "xT": hd["xT_all"][c],
            "idx": hd["idx_all"][c],
            "rel": hd["rel_all"][c],
            "gid": hd["gid_all"][c],
            "w1aug": hd["W1aug"],
            "w2aug": hd["W2aug"],
            "b1rep": b1rep,
            "b2rep": b2rep,
            "lin_w": np.asarray(inputs["lin_w"], np.float32),
            "lin_b": np.asarray(inputs["lin_b"], np.float32)[:, None],
            "cls_w": np.asarray(inputs["cls_w"], np.float32),
            "cls_b": np.asarray(inputs["cls_b"], np.float32)[:, None],
            "iota16": iota16,
            "ident16": ident16,
            "ident32": ident32,
            "ones16": ones16,
        })

    res = run_bass_kernel_spmd(nc, in_maps, core_ids=list(range(ncores)),
                               trace=trace)
    out = res.results[0]["out"]
    return out, res


def kernel(**inputs):
    out, _ = run_gnn(inputs, ncores=8)
    return out.astype(np.float32)


# revision 24
# speedup vs baseline: 2.6574x; 1.1226x over previous
"""GAT (2-layer) + global mean pool + MLP + log_softmax on 8 Trainium2 cores.

Strategy (dst-sharded message passing, bulk-gather edition):
  - Nodes partitioned across 8 cores; per-core node tables
    ([h@W | 1 | as | ad] as 128-col f16 rows = 256B) are computed shard-wise
    and replicated via AllGather into DRAM tables.
  - Per-edge h[src] rows are fetched with Pool-engine dma_gather (<=1024
    indices per call - the ucode limit), one call per (dst-block, quarter),
    rotated over 4 SWDGE queues (descriptor generation parallelizes ~3.6x
    across queues).  The table is addressed in 4 quarters so indices fit
    int16.
  - Local nodes are bin-packed into blocks balancing per-(block, quarter)
    edge counts, so every bucket fits tpq tiles of 128 edges with minimal
    padding.  Self loops bypass the gather entirely (local rows, identity
    one-hot).
  - Per-edge attention weights use exp(leakyrelu(x)) = max(exp(x),
    exp(0.2x)): two Exp passes on the otherwise-idle scalar engine (table
    pinned to Exp), so the vector engine only builds the one-hot, one add
    and one max per block.
  - Aggregation is a PSUM-accumulated one-hot matmul per 128-edge tile with
    the softmax denominator riding along as the table's constant-1 column.
  - Graph pooling = one-hot matmul + AllReduce; tiny MLP + log_softmax run
    redundantly on every core.
"""

import sys

sys.path.insert(0, "/opt/trn_rl_repo")

import numpy as np

P = 128
NQ = 4          # src-quarters (int16 index range per dma_gather)
ROW = 128       # table row width in f16 elems (256B, dma_gather granularity)
HID = 64
ONEC = HID      # constant-1 column (denominator rides the matmul)
ASC = HID + 1   # alpha_src column
ADC = HID + 2   # alpha_dst column
TW = HID + 3    # populated row prefix
RW = HID + 1    # rhs width for the aggregation matmul: [h | 1]
CHUNK = 10      # dst blocks per gather chunk


def _pack_blocks(deg_q, nblk, cap):
    """First-fit-decreasing bin packing: assign nodes (rows of deg_q
    [npc, NQ]) to nblk bins with <= P nodes per bin and per-quarter edge
    count <= cap.  Returns pos[npc] (slot b*P + i) or None."""
    npc = deg_q.shape[0]
    order = np.argsort(-deg_q.sum(1), kind="stable")
    rem = np.full((nblk, NQ), cap, np.int64)
    cnt = np.zeros(nblk, np.int64)
    pos = np.empty(npc, np.int64)
    for l in order:
        ok = (cnt < P) & (rem >= deg_q[l]).all(1)
        b = int(np.argmax(ok))
        if not ok[b]:
            return None
        pos[l] = b * P + cnt[b]
        cnt[b] += 1
        rem[b] -= deg_q[l]
    return pos


def _build_host_data(x, edge_index, batch, W1, a_src1, a_dst1, W2, a_src2,
                     a_dst2, ncores):
    """Pure-integer/graph preprocessing + augmented weights (host side)."""
    N, F_IN = x.shape
    assert N % ncores == 0 and ncores % NQ == 0
    npc = N // ncores
    nblk = -(-npc // P)
    chunk = min(CHUNK, nblk)
    nblk = -(-nblk // chunk) * chunk
    npc_pad = nblk * P
    V = ncores * npc_pad
    qrows = V // NQ
    assert qrows <= 32767
    nchunk = nblk // chunk

    # self loops are handled separately on-device (local rows, no gather)
    src = np.asarray(edge_index[0])
    dst = np.asarray(edge_index[1])
    score = (src // npc).astype(np.int64)
    dcore = (dst // npc).astype(np.int64)
    sloc = (src % npc).astype(np.int64)
    dloc = (dst % npc).astype(np.int64)

    # per-node quarter in-degree, then degree-balanced packing into blocks
    deg = np.zeros((ncores, npc, NQ), np.int64)
    # quarter of the src depends on its packed position; quarters span whole
    # cores (qrows is a multiple of npc_pad * cores-per-quarter), so the
    # quarter is known before packing:
    cpq = ncores // NQ
    q_of = score // cpq
    np.add.at(deg, (dcore, dloc, q_of), 1)

    # choose (tiles-per-quarter cap, block count) minimizing total tiles;
    # extra blocks buy packing slack that lets a lower cap succeed
    pos_all = np.empty((ncores, npc), np.int64)
    tpq = None
    opts = [(cap, nb)
            for cap in range(max(1, -(-deg.sum(2).max() // P)), 9)
            for nb in (nblk, nblk + chunk, nblk + 2 * chunk)
            if ncores * nb * P // NQ <= 32767 or nb == nblk]
    opts.sort(key=lambda o: o[1] * (NQ * o[0] + 1))
    qload = deg.sum(1)  # [ncores, NQ]
    for cap_tiles, nb in opts:
        if qload.max() > nb * cap_tiles * P:
            continue
        ok = True
        for c in range(ncores):
            pos = _pack_blocks(deg[c], nb, cap_tiles * P)
            if pos is None:
                ok = False
                break
            pos_all[c] = pos
        if ok:
            tpq, nblk = cap_tiles, nb
            break
    assert tpq is not None
    assert tpq * P <= 1024  # dma_gather ucode faults above 1024 indices
    npc_pad = nblk * P
    V = ncores * npc_pad
    qrows = V // NQ
    assert qrows <= 32767
    nchunk = nblk // chunk
    tpb = NQ * tpq

    srow = score * npc_pad + pos_all[score, sloc]
    qoff = (srow % qrows).astype(np.int64)
    dpos = pos_all[dcore, dloc]
    blk = dpos // P
    rel = dpos % P

    cnt = np.zeros((ncores, nblk, NQ), np.int64)
    np.add.at(cnt, (dcore, blk, q_of), 1)
    tpq = int(max(1, -(-cnt.max() // P)))
    tpb = NQ * tpq

    call_idx = np.zeros((ncores, NQ, nblk, tpq * P), np.int16)
    rel_all = np.full((ncores, P, nblk * tpb), -1.0, np.float16)

    order = np.lexsort((q_of, blk, dcore))
    so_q, so_b, so_c = q_of[order], blk[order], dcore[order]
    so_qoff, so_rel = qoff[order], rel[order]
    key = (so_c * nblk + so_b) * NQ + so_q
    start = np.searchsorted(key, np.arange(ncores * nblk * NQ), side="left")
    pos = np.arange(len(key)) - start[key]
    call_idx[so_c, so_q, so_b, pos] = so_qoff.astype(np.int16)
    rel_col = so_b * tpb + so_q * tpq + pos // P
    rel_all[so_c, pos % P, rel_col] = so_rel.astype(np.float16)

    # wrap indices for the gpsimd cores: idx j lives at [p % 16 == j % 16,
    # j // 16], replicated across the 8 groups of 16 partitions
    ci = call_idx.reshape(ncores, NQ, nblk, -1, 16)
    ci = np.transpose(ci, (0, 1, 2, 4, 3))
    ci = np.broadcast_to(ci[:, :, :, None, :, :],
                         (ncores, NQ, nblk, 8, 16, tpq * P // 16))
    idx_all = ci.reshape(ncores, NQ, nblk, P, -1)
    idx_all = np.transpose(idx_all, (0, 3, 1, 2, 4)).reshape(ncores, P, -1)
    idx_all = np.ascontiguousarray(idx_all)

    gid_all = np.full((ncores, P, nblk), -1.0, np.float32)
    xT_all = np.zeros((ncores, F_IN, npc_pad), np.float16)
    for c in range(ncores):
        ids = np.arange(npc)
        gg = np.full(npc_pad, -1.0, np.float32)
        gg[pos_all[c]] = batch[ids + c * npc].astype(np.float32)
        gid_all[c] = gg.reshape(nblk, P).T
        xT_all[c][:, pos_all[c]] = x[c * npc:(c + 1) * npc].T.astype(
            np.float16)

    def aug(W, a_s, a_d):
        w = np.zeros((W.shape[0], TW), np.float32)
        w[:, :HID] = W
        w[:, ASC] = W @ a_s
        w[:, ADC] = W @ a_d
        return w.astype(np.float16)

    return dict(npc=npc, nblk=nblk, npc_pad=npc_pad, tpq=tpq, tpb=tpb, V=V,
                chunk=chunk, qrows=qrows, nchunk=nchunk, idx_all=idx_all,
                rel_all=rel_all, gid_all=gid_all, xT_all=xT_all,
                W1aug=aug(W1, a_src1, a_dst1), W2aug=aug(W2, a_src2, a_dst2))


def _build_program(ncores, nblk, tpq, F_IN, G, C, V, qrows, nchunk, chunk):
    import concourse.bass as bass
    import concourse.bacc as bacc
    import concourse.tile as tile
    from concourse import mybir

    tpb = NQ * tpq
    npc_pad = nblk * P
    nidx = tpq * P                    # indices per (block, quarter) gather
    idxw = nidx // 16                 # idx cols per call (int16, wrapped)

    nc = bacc.Bacc("TRN2", target_bir_lowering=False, debug=False,
                   num_devices=ncores, num_swdge_queues=4)
    f32, f16, i16 = mybir.dt.float32, mybir.dt.float16, mybir.dt.int16
    Alu = mybir.AluOpType
    Act = mybir.ActivationFunctionType

    ein = lambda n, s, d: nc.dram_tensor(n, s, d, kind="ExternalInput")
    xT_d = ein("xT", [F_IN, npc_pad], f16)
    idx_d = ein("idx", [P, NQ * nblk * idxw], i16)
    rel_d = ein("rel", [P, nblk * tpb], f16)
    gid_d = ein("gid", [P, nblk], f32)
    w1_d = ein("w1aug", [F_IN, TW], f16)
    w2_d = ein("w2aug", [HID, TW], f16)
    b1_d = ein("b1rep", [P, HID], f32)
    b2_d = ein("b2rep", [P, HID], f32)
    lw_d = ein("lin_w", [HID, HID // 2], f32)
    lb_d = ein("lin_b", [HID // 2, 1], f32)
    cw_d = ein("cls_w", [HID // 2, C], f32)
    cb_d = ein("cls_b", [C, 1], f32)
    io16_d = ein("iota16", [P, P], f16)
    id16_d = ein("ident16", [P, P], f16)
    id32_d = ein("ident32", [P, P], f32)
    one16_d = ein("ones16", [P, 1], f16)
    out_d = nc.dram_tensor("out", [G, C], f32, kind="ExternalOutput")

    table1 = nc.dram_tensor("table1", [V, ROW], f16, kind="Internal")
    table2 = nc.dram_tensor("table2", [V, ROW], f16, kind="Internal")

    with tile.TileContext(nc) as tc:
        with (
            tc.tile_pool(name="cst", bufs=1) as cst,
            tc.tile_pool(name="sb", bufs=3) as sb,
            tc.tile_pool(name="gat", bufs=2) as gat,
            tc.tile_pool(name="ps", bufs=4, space="PSUM") as ps,
            tc.tile_pool(name="psacc", bufs=3, space="PSUM") as psacc,
            tc.tile_pool(name="pspool", bufs=1, space="PSUM") as pspool,
            tc.tile_pool(name="dram", bufs=1, space="DRAM") as dram,
        ):
            # ---- constants ----
            rel_t = cst.tile([P, nblk * tpb], f16)
            nc.sync.dma_start(rel_t[:], rel_d[:, :])
            gid_t = cst.tile([P, nblk], f32)
            nc.sync.dma_start(gid_t[:], gid_d[:, :])
            w1_t = cst.tile([F_IN, TW], f16)
            nc.sync.dma_start(w1_t[:], w1_d[:, :])
            w2_t = cst.tile([HID, TW], f16)
            nc.sync.dma_start(w2_t[:], w2_d[:, :])
            b1_t = cst.tile([P, HID], f32)
            nc.sync.dma_start(b1_t[:], b1_d[:, :])
            b2_t = cst.tile([P, HID], f32)
            nc.sync.dma_start(b2_t[:], b2_d[:, :])
            io16_t = cst.tile([P, P], f16)
            nc.sync.dma_start(io16_t[:], io16_d[:, :])
            id16_t = cst.tile([P, P], f16)
            nc.sync.dma_start(id16_t[:], id16_d[:, :])
            id32_t = cst.tile([P, P], f32)
            nc.sync.dma_start(id32_t[:], id32_d[:, :])
            one16_t = cst.tile([P, 1], f16)
            nc.sync.dma_start(one16_t[:], one16_d[:, :])
            xT_t = cst.tile([F_IN, npc_pad], f16)
            nc.sync.dma_start(xT_t[:], xT_d[:, :])

            slice1 = dram.tile([npc_pad, ROW], f16)
            slice2 = dram.tile([npc_pad, ROW], f16)
            pool_in = dram.tile([P, HID + 1], f32)
            pool_out = dram.tile([P, HID + 1], f32)

            pooled_ps = pspool.tile([P, HID + 1], f32)

            io_b = io16_t[:].rearrange("p (u v) -> p u v", u=1).to_broadcast(
                [P, tpb, P])
            qcall = [0]

            # ---- phase 0: slice1 rows = [x@W1 | 1 | as1 | ad1] ----
            for b in range(nblk):
                t1T_ps = ps.tile([TW, P], f32, tag="pst")
                nc.tensor.matmul(t1T_ps[:], lhsT=w1_t[:],
                                 rhs=xT_t[:, b * P:(b + 1) * P],
                                 start=True, stop=True)
                t1T_sb = sb.tile([TW, P], f16, tag="t1Tsb")
                nc.vector.tensor_copy(t1T_sb[:], t1T_ps[:])
                t1_ps = ps.tile([P, TW], f16, tag="pst")
                nc.tensor.transpose(t1_ps[:], t1T_sb[:], id16_t[:TW, :TW])
                t1_sb = sb.tile([P, TW], f16, tag="t1sb")
                nc.vector.tensor_copy(t1_sb[:], t1_ps[:])
                nc.vector.memset(t1_sb[:, ONEC:ONEC + 1], 1.0)
                nc.sync.dma_start(slice1[b * P:(b + 1) * P, 0:TW], t1_sb[:])

            nc.gpsimd.collective_compute(
                "AllGather", Alu.bypass,
                replica_groups=[list(range(ncores))],
                ins=[slice1.opt()], outs=[table1[:, :]],
            )

            def gat_layer(table_h, slice_ap, is_last):
                lname = "L2" if is_last else "L1"
                # ad[dst] for local nodes: ad_grid[p, b] = slice[b*128+p, ADC]
                ad_grid = cst.tile([P, nblk], f16, name=f"adg{lname}")
                nc.sync.dma_start(
                    ad_grid[:],
                    slice_ap[:, ADC:ADC + 1].rearrange(
                        "(b p) c -> p (b c)", p=P),
                )
                for ch in range(nchunk):
                    gq3 = []
                    for q in range(NQ):
                        idxq = sb.tile([P, chunk * idxw], i16, tag=f"idx{q}")
                        nc.sync.dma_start(
                            idxq[:],
                            idx_d[:, (q * nblk + ch * chunk) * idxw:
                                  (q * nblk + (ch + 1) * chunk) * idxw])
                        g = gat.tile([P, chunk * tpq * ROW], f16,
                                     tag=f"g{q}")
                        g3 = g[:].rearrange("p (c e) -> p c e", e=ROW)
                        # pack as many blocks per call as the 1024-index
                        # ucode limit allows
                        bpc = max(1, 1024 // nidx)
                        while chunk % bpc:
                            bpc -= 1
                        for j in range(0, chunk, bpc):
                            nc.gpsimd.dma_gather(
                                out_ap=g3[:, j * tpq:(j + bpc) * tpq, :],
                                in_ap=table_h[q * qrows:(q + 1) * qrows, :],
                                idxs_ap=idxq[:, j * idxw:(j + bpc) * idxw],
                                num_idxs=nidx * bpc,
                                num_idxs_reg=nidx * bpc,
                                elem_size=ROW,
                                queue_num=qcall[0] % 4,
                            )
                            qcall[0] += 1
                        gq3.append(g3)
                    # self-loop rows of this chunk's blocks (local, seq DMA)
                    sf = gat.tile([P, chunk * ROW], f16, tag="self")
                    nc.sync.dma_start(
                        sf[:].rearrange("p (b e) -> p b e", e=ROW),
                        slice_ap[ch * chunk * P:(ch + 1) * chunk * P,
                                 :].rearrange("(b p) e -> p b e", p=P))

                    for j in range(chunk):
                        b = ch * chunk + j
                        # adR[p, v] = ad of dst v in this block
                        ad_blk = sb.tile([P, 1], f16, tag="adblk")
                        nc.vector.tensor_copy(ad_blk[:],
                                              ad_grid[:, b:b + 1])
                        adR_ps = ps.tile([P, P], f16, tag="pst")
                        nc.tensor.transpose(
                            adR_ps[:], ad_blk[:, 0:1].to_broadcast([P, P]),
                            id16_t[:])
                        adR = sb.tile([P, P], f16, tag="adR")
                        nc.vector.tensor_copy(adR[:], adR_ps[:])
                        # one-hot S over all tiles of the block
                        S_all = sb.tile([P, tpb * P], f16, tag="S")
                        nc.vector.tensor_tensor(
                            out=S_all[:].rearrange("p (t v) -> p t v", v=P),
                            in0=rel_t[:, b * tpb:(b + 1) * tpb].rearrange(
                                "p (t u) -> p t u", u=1).to_broadcast(
                                [P, tpb, P]),
                            in1=io_b, op=Alu.is_equal)
                        # X[p,t,v] = ad[v] + as[p,t], as read straight from
                        # the gathered rows (col ASC), one op per quarter
                        X_all = sb.tile([P, tpb * P], f16, tag="X")
                        adR_b1 = adR[:].rearrange(
                            "p (u v) -> p u v", u=1).to_broadcast(
                            [P, tpq, P])
                        for q in range(NQ):
                            nc.vector.tensor_tensor(
                                out=X_all[:, q * tpq * P:
                                          (q + 1) * tpq * P].rearrange(
                                    "p (t v) -> p t v", v=P),
                                in0=adR_b1,
                                in1=gq3[q][:, j * tpq:(j + 1) * tpq,
                                           ASC:ASC + 1].to_broadcast(
                                    [P, tpq, P]),
                                op=Alu.add)
                        # exp(leakyrelu(x)) = max(exp(x), exp(0.2 x));
                        # both Exp -> no activation-table thrash
                        E2 = sb.tile([P, tpb * P], f16, tag="E2")
                        nc.scalar.activation(E2[:], X_all[:], Act.Exp,
                                             scale=0.2)
                        nc.scalar.activation(X_all[:], X_all[:], Act.Exp)
                        nc.vector.tensor_tensor(out=X_all[:], in0=X_all[:],
                                                in1=E2[:], op=Alu.max)
                        nc.vector.tensor_tensor(out=S_all[:], in0=S_all[:],
                                                in1=X_all[:], op=Alu.mult)
                        # self loop weight from the local row
                        xes = sb.tile([P, 1], f32, tag="xes")
                        nc.vector.tensor_tensor(
                            out=xes[:], in0=sf[:, j * ROW + ASC:
                                               j * ROW + ASC + 1],
                            in1=sf[:, j * ROW + ADC:j * ROW + ADC + 1],
                            op=Alu.add)
                        e2s = sb.tile([P, 1], f32, tag="e2s")
                        nc.scalar.activation(e2s[:], xes[:], Act.Exp,
                                             scale=0.2)
                        nc.scalar.activation(xes[:], xes[:], Act.Exp)
                        nc.vector.tensor_tensor(out=xes[:], in0=xes[:],
                                                in1=e2s[:], op=Alu.max)
                        exSs = sb.tile([P, P], f16, tag="exSs")
                        nc.vector.tensor_tensor(
                            out=exSs[:], in0=id16_t[:],
                            in1=xes[:].to_broadcast([P, P]), op=Alu.mult)

                        acc = psacc.tile([P, RW], f32, tag="acc")
                        for q in range(NQ):
                            for i in range(tpq):
                                t = q * tpq + i
                                nc.tensor.matmul(
                                    acc[:],
                                    lhsT=S_all[:, t * P:(t + 1) * P],
                                    rhs=gq3[q][:, j * tpq + i:
                                               j * tpq + i + 1,
                                               0:RW].rearrange(
                                        "p c e -> p (c e)"),
                                    start=(t == 0), stop=False)
                        nc.tensor.matmul(
                            acc[:], lhsT=exSs[:],
                            rhs=sf[:, j * ROW:j * ROW + RW],
                            start=False, stop=True)

                        den = sb.tile([P, 1], f32, tag="den")
                        nc.vector.tensor_scalar(
                            out=den[:], in0=acc[:, HID:HID + 1],
                            scalar1=1e-30, scalar2=None, op0=Alu.max)
                        rec = sb.tile([P, 1], f32, tag="rec")
                        nc.vector.reciprocal(rec[:], den[:])
                        hv = sb.tile([P, HID], f32, tag="hv")
                        nc.vector.tensor_tensor(
                            out=hv[:], in0=acc[:, 0:HID],
                            in1=rec[:].to_broadcast([P, HID]), op=Alu.mult)
                        if not is_last:
                            nc.vector.tensor_tensor(out=hv[:], in0=hv[:],
                                                    in1=b1_t[:], op=Alu.add)
                            nc.vector.tensor_scalar(out=hv[:], in0=hv[:],
                                                    scalar1=0.0, scalar2=None,
                                                    op0=Alu.max)
                            hv16 = sb.tile([P, HID], f16, tag="hv16")
                            nc.vector.tensor_copy(hv16[:], hv[:])
                            hvT_ps = ps.tile([HID, P], f16, tag="pst")
                            nc.tensor.transpose(hvT_ps[:], hv16[:], id16_t[:])
                            hvT = sb.tile([HID, P], f16, tag="hvT")
                            nc.vector.tensor_copy(hvT[:], hvT_ps[:])
                            t2T_ps = ps.tile([TW, P], f32, tag="pst")
                            nc.tensor.matmul(t2T_ps[:], lhsT=w2_t[:],
                                             rhs=hvT[:], start=True,
                                             stop=True)
                            t2T_sb = sb.tile([TW, P], f16, tag="t2Tsb")
                            nc.vector.tensor_copy(t2T_sb[:], t2T_ps[:])
                            t2_ps = ps.tile([P, TW], f16, tag="pst")
                            nc.tensor.transpose(t2_ps[:], t2T_sb[:],
                                                id16_t[:TW, :TW])
                            t2_sb = sb.tile([P, TW], f16, tag="t2sb")
                            nc.vector.tensor_copy(t2_sb[:], t2_ps[:])
                            nc.vector.memset(t2_sb[:, ONEC:ONEC + 1], 1.0)
                            nc.sync.dma_start(
                                slice2[b * P:(b + 1) * P, 0:TW], t2_sb[:])
                        else:
                            nc.vector.tensor_tensor(out=hv[:], in0=hv[:],
                                                    in1=b2_t[:], op=Alu.add)
                            prhs = sb.tile([P, HID + 1], f16, tag="prhs")
                            nc.vector.tensor_copy(prhs[:, 0:HID], hv[:])
                            nc.vector.tensor_copy(prhs[:, HID:HID + 1],
                                                  one16_t[:])
                            gid_col = sb.tile([P, 1], f16, tag="gidcol")
                            nc.vector.tensor_copy(gid_col[:],
                                                  gid_t[:, b:b + 1])
                            Gh = sb.tile([P, P], f16, tag="Gh")
                            nc.vector.tensor_tensor(
                                out=Gh[:],
                                in0=gid_col[:].to_broadcast([P, P]),
                                in1=io16_t[:], op=Alu.is_equal)
                            nc.tensor.matmul(pooled_ps[:], lhsT=Gh[:],
                                             rhs=prhs[:], start=(b == 0),
                                             stop=(b == nblk - 1))

            gat_layer(table1, slice1, is_last=False)
            nc.gpsimd.collective_compute(
                "AllGather", Alu.bypass,
                replica_groups=[list(range(ncores))],
                ins=[slice2.opt()], outs=[table2[:, :]],
            )
            gat_layer(table2, slice2, is_last=True)

            # ---- AllReduce pooled sums ----
            pooled_sb = sb.tile([P, HID + 1], f32, tag="pooledsb")
            nc.vector.tensor_copy(pooled_sb[:], pooled_ps[:])
            nc.sync.dma_start(pool_in[:, :], pooled_sb[:])
            nc.gpsimd.collective_compute(
                "AllReduce", Alu.add,
                replica_groups=[list(range(ncores))],
                ins=[pool_in.opt()], outs=[pool_out.opt()],
            )
            pl = sb.tile([P, HID + 1], f32, tag="pl")
            nc.sync.dma_start(pl[:], pool_out[:, :])

            cnt = sb.tile([P, 1], f32, tag="cnt")
            nc.vector.tensor_scalar(out=cnt[:], in0=pl[:, HID:HID + 1],
                                    scalar1=1.0, scalar2=None, op0=Alu.max)
            crec = sb.tile([P, 1], f32, tag="crec")
            nc.vector.reciprocal(crec[:], cnt[:])
            mean = sb.tile([P, HID], f32, tag="mean")
            nc.vector.tensor_tensor(out=mean[:], in0=pl[:, 0:HID],
                                    in1=crec[:].to_broadcast([P, HID]),
                                    op=Alu.mult)

            # MLP: z = relu(mean @ lin_w + lin_b); logits = z @ cls_w + cls_b
            lw_t = cst.tile([HID, HID // 2], f32)
            nc.sync.dma_start(lw_t[:], lw_d[:, :])
            lb_t = cst.tile([HID // 2, 1], f32)
            nc.sync.dma_start(lb_t[:], lb_d[:, :])
            cw_t = cst.tile([HID // 2, C], f32)
            nc.sync.dma_start(cw_t[:], cw_d[:, :])
            cb_t = cst.tile([C, 1], f32)
            nc.sync.dma_start(cb_t[:], cb_d[:, :])

            meanT_ps = ps.tile([HID, P], f32, tag="pst")
            nc.tensor.transpose(meanT_ps[:], mean[:], id32_t[:])
            meanT = sb.tile([HID, P], f32, tag="meanT")
            nc.vector.tensor_copy(meanT[:], meanT_ps[:])
            zT_ps = ps.tile([HID // 2, P], f32, tag="pst")
            nc.tensor.matmul(zT_ps[:], lhsT=lw_t[:], rhs=meanT[:],
                             start=True, stop=True)
            zT = sb.tile([HID // 2, P], f32, tag="zT")
            nc.scalar.activation(zT[:], zT_ps[:], Act.Relu, bias=lb_t[:])
            lgT_ps = ps.tile([C, P], f32, tag="pst")
            nc.tensor.matmul(lgT_ps[:], lhsT=cw_t[:], rhs=zT[:],
                             start=True, stop=True)
            lgT = sb.tile([C, P], f32, tag="lgT")
            nc.scalar.activation(lgT[:], lgT_ps[:], Act.Identity, bias=cb_t[:])
            lg_ps = ps.tile([P, C], f32, tag="pst")
            nc.tensor.transpose(lg_ps[:], lgT[:], id32_t[:C, :C])
            lg = sb.tile([P, C], f32, tag="lg")
            nc.vector.tensor_copy(lg[:], lg_ps[:])

            mx = sb.tile([P, 1], f32, tag="mx")
            nc.vector.tensor_reduce(mx[:], lg[:], axis=mybir.AxisListType.X,
                                    op=Alu.max)
            sh = sb.tile([P, C], f32, tag="sh")
            nc.vector.tensor_tensor(out=sh[:], in0=lg[:],
                                    in1=mx[:].to_broadcast([P, C]),
                                    op=Alu.subtract)
            exs = sb.tile([P, C], f32, tag="exs")
            se = sb.tile([P, 1], f32, tag="se")
            nc.scalar.activation(exs[:], sh[:], Act.Exp, accum_out=se[:])
            lse = sb.tile([P, 1], f32, tag="lse")
            nc.scalar.activation(lse[:], se[:], Act.Ln)
            res = sb.tile([P, C], f32, tag="res")
            nc.vector.tensor_tensor(out=res[:], in0=sh[:],
                                    in1=lse[:].to_broadcast([P, C]),
                                    op=Alu.subtract)
            nc.sync.dma_start(out_d[:, :], res[:])

    nc.compile()
    return nc


def run_gnn(inputs, ncores=8, trace=False):
    from concourse.bass_utils import run_bass_kernel_spmd

    x = np.asarray(inputs["x"], np.float32)
    edge_index = np.asarray(inputs["edge_index"])
    batch = np.asarray(inputs["batch"])
    W1 = np.asarray(inputs["W1"], np.float32)
    W2 = np.asarray(inputs["W2"], np.float32)
    hd = _build_host_data(
        x, edge_index, batch, W1,
        np.asarray(inputs["a_src1"], np.float32),
        np.asarray(inputs["a_dst1"], np.float32),
        W2,
        np.asarray(inputs["a_src2"], np.float32),
        np.asarray(inputs["a_dst2"], np.float32),
        ncores)

    N, F_IN = x.shape
    G = 128  # number of graphs == P (pooling one-hot relies on this)
    C = np.asarray(inputs["cls_w"]).shape[1]

    nc = _build_program(ncores, hd["nblk"], hd["tpq"], F_IN, G, C,
                        hd["V"], hd["qrows"], hd["nchunk"], hd["chunk"])

    iota16 = np.tile(np.arange(P, dtype=np.float16)[None, :], (P, 1))
    ident16 = np.eye(P, dtype=np.float16)
    ident32 = np.eye(P, dtype=np.float32)
    ones16 = np.ones((P, 1), np.float16)
    b1rep = np.tile(np.asarray(inputs["b1"], np.float32)[None, :], (P, 1))
    b2rep = np.tile(np.asarray(inputs["b2"], np.float32)[None, :], (P, 1))

    in_maps = []
    for c in range(ncores):
        in_maps.append({
            "xT": hd["xT_all"][c],
            "idx": hd["idx_all"][c],
            "rel": hd["rel_all"][c],
            "gid": hd["gid_all"][c],
            "w1aug": hd["W1aug"],
            "w2aug": hd["W2aug"],
            "b1rep": b1rep,
            "b2rep": b2rep,
            "lin_w": np.asarray(inputs["lin_w"], np.float32),
            "lin_b": np.asarray(inputs["lin_b"], np.float32)[:, None],
            "cls_w": np.asarray(inputs["cls_w"], np.float32),
            "cls_b": np.asarray(inputs["cls_b"], np.float32)[:, None],
            "iota16": iota16,
            "ident16": ident16,
            "ident32": ident32,
            "ones16": ones16,
        })

    res = run_bass_kernel_spmd(nc, in_maps, core_ids=list(range(ncores)),
                               trace=trace)
    out = res.results[0]["out"]
    return out, res


def kernel(**inputs):
    out, _ = run_gnn(inputs, ncores=8)
    return out.astype(np.float32)


# revision 30
# speedup vs baseline: 3.8195x; 1.4373x over previous
"""GAT (2-layer) + global mean pool + MLP + log_softmax on 8 Trainium2 cores.

Strategy (dst-sharded message passing, bulk-gather edition):
  - Nodes partitioned across 8 cores; per-core node tables
    ([h@W | 1 | as | ad] as 128-col f16 rows = 256B) are computed shard-wise
    and replicated via AllGather into DRAM tables.
  - Per-edge h[src] rows are fetched with Pool-engine dma_gather (<=1024
    indices per call - the ucode limit), one call per (dst-block, quarter),
    rotated over 4 SWDGE queues (descriptor generation parallelizes ~3.6x
    across queues).  The table is addressed in 4 quarters so indices fit
    int16.
  - Local nodes are bin-packed into blocks balancing per-(block, quarter)
    edge counts, so every bucket fits tpq tiles of 128 edges with minimal
    padding.  Self loops bypass the gather entirely (local rows, identity
    one-hot).
  - Per-edge attention weights use exp(leakyrelu(x)) = max(exp(x),
    exp(0.2x)): two Exp passes on the otherwise-idle scalar engine (table
    pinned to Exp), so the vector engine only builds the one-hot, one add
    and one max per block.
  - Aggregation is a PSUM-accumulated one-hot matmul per 128-edge tile with
    the softmax denominator riding along as the table's constant-1 column.
  - Graph pooling = one-hot matmul + AllReduce; tiny MLP + log_softmax run
    redundantly on every core.
"""

import sys

sys.path.insert(0, "/opt/trn_rl_repo")

import numpy as np

P = 128
NQ = 4          # src-quarters (int16 index range per dma_gather)
ROW = 128       # table row width in f16 elems (256B, dma_gather granularity)
HID = 64
ONEC = HID      # constant-1 column (denominator rides the matmul)
ASC = HID + 1   # alpha_src column
ADC = HID + 2   # alpha_dst column
TW = HID + 3    # populated row prefix
RW = HID + 1    # rhs width for the aggregation matmul: [h | 1]
CHUNK = 10      # dst blocks per gather chunk


def _pack_blocks(deg_q, nblk, cap):
    """First-fit-decreasing bin packing: assign nodes (rows of deg_q
    [npc, NQ]) to nblk bins with <= P nodes per bin and per-quarter edge
    count <= cap.  Returns pos[npc] (slot b*P + i) or None."""
    npc = deg_q.shape[0]
    order = np.argsort(-deg_q.sum(1), kind="stable")
    rem = np.full((nblk, NQ), cap, np.int64)
    cnt = np.zeros(nblk, np.int64)
    pos = np.empty(npc, np.int64)
    for l in order:
        ok = (cnt < P) & (rem >= deg_q[l]).all(1)
        b = int(np.argmax(ok))
        if not ok[b]:
            return None
        pos[l] = b * P + cnt[b]
        cnt[b] += 1
        rem[b] -= deg_q[l]
    return pos


def _build_host_data(x, edge_index, batch, W1, a_src1, a_dst1, W2, a_src2,
                     a_dst2, ncores):
    """Pure-integer/graph preprocessing + augmented weights (host side)."""
    N, F_IN = x.shape
    assert N % ncores == 0 and ncores % NQ == 0
    npc = N // ncores
    assert npc % NQ == 0
    npg = npc // NQ  # nodes per (core, group)

    # self loops are handled separately on-device (local rows, no gather)
    src = np.asarray(edge_index[0])
    dst = np.asarray(edge_index[1])
    score = (src // npc).astype(np.int64)
    dcore = (dst // npc).astype(np.int64)
    sloc = (src % npc).astype(np.int64)
    dloc = (dst % npc).astype(np.int64)

    # the table is laid out in NQ segments; segment q holds every core's
    # q-th node group, so a node's int16-index quarter equals its group
    # (known before packing) and the table AllGather can be pipelined as
    # NQ sub-collectives
    q_of = sloc // npg

    # per-node quarter in-degree, then degree-balanced packing into blocks
    deg = np.zeros((ncores, npc, NQ), np.int64)
    np.add.at(deg, (dcore, dloc, q_of), 1)

    # choose (tiles-per-quarter cap, per-group block count) minimizing total
    # tiles; extra blocks buy packing slack that lets a lower cap succeed
    nb4_base = -(-npg // P)
    pos_all = np.empty((ncores, npc), np.int64)
    tpq = None
    opts = [(cap, nb4)
            for cap in range(1, 9)
            for nb4 in (nb4_base, nb4_base + 1, nb4_base + 2, nb4_base + 3)
            if ncores * nb4 * P <= 32767 or nb4 == nb4_base]
    opts.sort(key=lambda o: o[1] * (NQ * o[0] + 1))
    qload = deg.sum(1)  # [ncores, NQ]
    gload = np.zeros((ncores, NQ, NQ), np.int64)  # [core, dst group, src q]
    np.add.at(gload, (dcore, dloc // npg, q_of), 1)
    for cap_tiles, nb4 in opts:
        if gload.max() > nb4 * cap_tiles * P:
            continue
        ok = True
        for c in range(ncores):
            for g in range(NQ):
                ids = np.arange(g * npg, (g + 1) * npg)
                pos = _pack_blocks(deg[c][ids], nb4, cap_tiles * P)
                if pos is None:
                    ok = False
                    break
                pos_all[c, ids] = g * nb4 * P + pos
            if not ok:
                break
        if ok:
            tpq = cap_tiles
            break
    assert tpq is not None
    assert tpq * P <= 1024  # dma_gather ucode faults above 1024 indices
    nblk = NQ * nb4
    npc_pad = nblk * P
    sub = nb4 * P           # rows per (core, segment)
    V = ncores * npc_pad
    qrows = V // NQ
    assert qrows <= 32767
    chunk = nb4
    nchunk = NQ
    tpb = NQ * tpq

    spos = pos_all[score, sloc]
    srow = (spos // sub) * qrows + score * sub + spos % sub
    qoff = (srow % qrows).astype(np.int64)
    dpos = pos_all[dcore, dloc]
    blk = dpos // P
    rel = dpos % P

    cnt = np.zeros((ncores, nblk, NQ), np.int64)
    np.add.at(cnt, (dcore, blk, q_of), 1)
    tpq = int(max(1, -(-cnt.max() // P)))
    tpb = NQ * tpq

    call_idx = np.zeros((ncores, NQ, nblk, tpq * P), np.int16)
    rel_all = np.full((ncores, P, nblk * tpb), -1.0, np.float16)

    order = np.lexsort((q_of, blk, dcore))
    so_q, so_b, so_c = q_of[order], blk[order], dcore[order]
    so_qoff, so_rel = qoff[order], rel[order]
    key = (so_c * nblk + so_b) * NQ + so_q
    start = np.searchsorted(key, np.arange(ncores * nblk * NQ), side="left")
    pos = np.arange(len(key)) - start[key]
    call_idx[so_c, so_q, so_b, pos] = so_qoff.astype(np.int16)
    rel_col = so_b * tpb + so_q * tpq + pos // P
    rel_all[so_c, pos % P, rel_col] = so_rel.astype(np.float16)

    # wrap indices for the gpsimd cores: idx j lives at [p % 16 == j % 16,
    # j // 16], replicated across the 8 groups of 16 partitions
    ci = call_idx.reshape(ncores, NQ, nblk, -1, 16)
    ci = np.transpose(ci, (0, 1, 2, 4, 3))
    ci = np.broadcast_to(ci[:, :, :, None, :, :],
                         (ncores, NQ, nblk, 8, 16, tpq * P // 16))
    idx_all = ci.reshape(ncores, NQ, nblk, P, -1)
    idx_all = np.transpose(idx_all, (0, 3, 1, 2, 4)).reshape(ncores, P, -1)
    idx_all = np.ascontiguousarray(idx_all)

    gid_all = np.full((ncores, P, nblk), -1.0, np.float32)
    xT_all = np.zeros((ncores, F_IN, npc_pad), np.float16)
    for c in range(ncores):
        ids = np.arange(npc)
        gg = np.full(npc_pad, -1.0, np.float32)
        gg[pos_all[c]] = batch[ids + c * npc].astype(np.float32)
        gid_all[c] = gg.reshape(nblk, P).T
        xT_all[c][:, pos_all[c]] = x[c * npc:(c + 1) * npc].T.astype(
            np.float16)

    def aug(W, a_s, a_d):
        w = np.zeros((W.shape[0], TW), np.float32)
        w[:, :HID] = W
        w[:, ASC] = W @ a_s
        w[:, ADC] = W @ a_d
        return w.astype(np.float16)

    return dict(npc=npc, nblk=nblk, npc_pad=npc_pad, tpq=tpq, tpb=tpb, V=V,
                chunk=chunk, qrows=qrows, nchunk=nchunk, idx_all=idx_all,
                rel_all=rel_all, gid_all=gid_all, xT_all=xT_all,
                W1aug=aug(W1, a_src1, a_dst1), W2aug=aug(W2, a_src2, a_dst2))


def _build_program(ncores, nblk, tpq, F_IN, G, C, V, qrows, nchunk, chunk):
    import concourse.bass as bass
    import concourse.bacc as bacc
    import concourse.tile as tile
    from concourse import mybir

    tpb = NQ * tpq
    npc_pad = nblk * P
    nidx = tpq * P                    # indices per (block, quarter) gather
    idxw = nidx // 16                 # idx cols per call (int16, wrapped)

    nc = bacc.Bacc("TRN2", target_bir_lowering=False, debug=False,
                   num_devices=ncores, num_swdge_queues=4)
    f32, f16, i16 = mybir.dt.float32, mybir.dt.float16, mybir.dt.int16
    Alu = mybir.AluOpType
    Act = mybir.ActivationFunctionType

    ein = lambda n, s, d: nc.dram_tensor(n, s, d, kind="ExternalInput")
    xT_d = ein("xT", [F_IN, npc_pad], f16)
    idx_d = ein("idx", [P, NQ * nblk * idxw], i16)
    rel_d = ein("rel", [P, nblk * tpb], f16)
    gid_d = ein("gid", [P, nblk], f32)
    w1_d = ein("w1aug", [F_IN, TW], f16)
    w2_d = ein("w2aug", [HID, TW], f16)
    b1_d = ein("b1rep", [P, HID], f32)
    b2_d = ein("b2rep", [P, HID], f32)
    lw_d = ein("lin_w", [HID, HID // 2], f32)
    lb_d = ein("lin_b", [HID // 2, 1], f32)
    cw_d = ein("cls_w", [HID // 2, C], f32)
    cb_d = ein("cls_b", [C, 1], f32)
    io16_d = ein("iota16", [P, P], f16)
    id16_d = ein("ident16", [P, P], f16)
    id32_d = ein("ident32", [P, P], f32)
    one16_d = ein("ones16", [P, 1], f16)
    out_d = nc.dram_tensor("out", [G, C], f32, kind="ExternalOutput")

    table1 = nc.dram_tensor("table1", [V, ROW], f16, kind="Internal")
    table2 = nc.dram_tensor("table2", [V, ROW], f16, kind="Internal")

    with tile.TileContext(nc) as tc:
        with (
            tc.tile_pool(name="cst", bufs=1) as cst,
            tc.tile_pool(name="sb", bufs=3) as sb,
            tc.tile_pool(name="sbt", bufs=6) as sbt,
            tc.tile_pool(name="gat", bufs=6) as gat,
            tc.tile_pool(name="ps", bufs=3, space="PSUM") as ps,
            tc.tile_pool(name="psa", bufs=2, space="PSUM") as psa,
            tc.tile_pool(name="psacc", bufs=2, space="PSUM") as psacc,
            tc.tile_pool(name="pspool", bufs=1, space="PSUM") as pspool,
            tc.tile_pool(name="dram", bufs=1, space="DRAM") as dram,
        ):
            # ---- constants ----
            rel_t = cst.tile([P, nblk * tpb], f16)
            nc.sync.dma_start(rel_t[:], rel_d[:, :])
            gid_t = cst.tile([P, nblk], f32)
            nc.sync.dma_start(gid_t[:], gid_d[:, :])
            w1_t = cst.tile([F_IN, TW], f16)
            nc.sync.dma_start(w1_t[:], w1_d[:, :])
            w2_t = cst.tile([HID, TW], f16)
            nc.sync.dma_start(w2_t[:], w2_d[:, :])
            b1_t = cst.tile([P, HID], f32)
            nc.sync.dma_start(b1_t[:], b1_d[:, :])
            b2_t = cst.tile([P, HID], f32)
            nc.sync.dma_start(b2_t[:], b2_d[:, :])
            io16_t = cst.tile([P, P], f16)
            nc.sync.dma_start(io16_t[:], io16_d[:, :])
            id16_t = cst.tile([P, P], f16)
            nc.sync.dma_start(id16_t[:], id16_d[:, :])
            id32_t = cst.tile([P, P], f32)
            nc.sync.dma_start(id32_t[:], id32_d[:, :])
            one16_t = cst.tile([P, 1], f16)
            nc.sync.dma_start(one16_t[:], one16_d[:, :])
            xT_t = cst.tile([F_IN, npc_pad], f16)
            nc.sync.dma_start(xT_t[:], xT_d[:, :])
            idx_t = cst.tile([P, NQ * nblk * idxw], i16)
            nc.sync.dma_start(idx_t[:], idx_d[:, :])

            slice1 = dram.tile([npc_pad, ROW], f16)
            slice2 = dram.tile([npc_pad, ROW], f16)
            pool_in = dram.tile([P, HID + 1], f32)
            pool_out = dram.tile([P, HID + 1], f32)

            pooled_ps = pspool.tile([P, HID + 1], f32)

            io_b = io16_t[:].rearrange("p (u v) -> p u v", u=1).to_broadcast(
                [P, tpb, P])
            qcall = [0]

            # ---- phase 0: slice1 rows = [x@W1 | 1 | as1 | ad1] ----
            for b in range(nblk):
                t1T_ps = ps.tile([TW, P], f32, tag="pst")
                nc.tensor.matmul(t1T_ps[:], lhsT=w1_t[:],
                                 rhs=xT_t[:, b * P:(b + 1) * P],
                                 start=True, stop=True)
                t1T_sb = sb.tile([TW, P], f16, tag="t1Tsb")
                nc.vector.tensor_copy(t1T_sb[:], t1T_ps[:])
                t1_ps = ps.tile([P, TW], f16, tag="pst")
                nc.tensor.transpose(t1_ps[:], t1T_sb[:], id16_t[:TW, :TW])
                t1_sb = sb.tile([P, TW], f16, tag="t1sb")
                nc.vector.tensor_copy(t1_sb[:], t1_ps[:])
                nc.vector.memset(t1_sb[:, ONEC:ONEC + 1], 1.0)
                nc.sync.dma_start(slice1[b * P:(b + 1) * P, 0:TW], t1_sb[:])

            sub = npc_pad // NQ
            for s in range(NQ):
                nc.gpsimd.collective_compute(
                    "AllGather", Alu.bypass,
                    replica_groups=[list(range(ncores))],
                    ins=[slice1[s * sub:(s + 1) * sub, :]],
                    outs=[table1[s * qrows:(s + 1) * qrows, :]],
                )

            def gat_layer(table_h, slice_ap, is_last):
                lname = "L2" if is_last else "L1"
                # ad[dst] for local nodes: ad_grid[p, b] = slice[b*128+p, ADC]
                ad_grid = cst.tile([P, nblk], f16, name=f"adg{lname}")
                nc.sync.dma_start(
                    ad_grid[:],
                    slice_ap[:, ADC:ADC + 1].rearrange(
                        "(b p) c -> p (b c)", p=P),
                )
                # blocks per gather call, bounded by the 1024-index ucode
                # limit; gathers pipeline at (bpc-blocks, quarter) grain
                bpc = max(1, 1024 // nidx)
                while nblk % bpc:
                    bpc -= 1
                for bp in range(nblk // bpc):
                    gq3 = []
                    for q in range(NQ):
                        g = gat.tile([P, bpc * tpq * ROW], f16, tag=f"g{q}")
                        g3 = g[:].rearrange("p (c e) -> p c e", e=ROW)
                        nc.gpsimd.dma_gather(
                            out_ap=g3,
                            in_ap=table_h[q * qrows:(q + 1) * qrows, :],
                            idxs_ap=idx_t[:, (q * nblk + bp * bpc) * idxw:
                                          (q * nblk + (bp + 1) * bpc) * idxw],
                            num_idxs=nidx * bpc,
                            num_idxs_reg=nidx * bpc,
                            elem_size=ROW,
                            queue_num=qcall[0] % 4,
                        )
                        qcall[0] += 1
                        gq3.append(g3)
                    # self-loop rows of these blocks (local, seq DMA)
                    sf = gat.tile([P, bpc * ROW], f16, tag="self")
                    nc.sync.dma_start(
                        sf[:].rearrange("p (b e) -> p b e", e=ROW),
                        slice_ap[bp * bpc * P:(bp + 1) * bpc * P,
                                 :].rearrange("(b p) e -> p b e", p=P))

                    for j in range(bpc):
                        b = bp * bpc + j
                        # adR[p, v] = ad of dst v in this block
                        ad_blk = sbt.tile([P, 1], f16, tag="adblk")
                        nc.vector.tensor_copy(ad_blk[:],
                                              ad_grid[:, b:b + 1])
                        adR_ps = psa.tile([P, P], f16, tag="adps")
                        nc.tensor.transpose(
                            adR_ps[:], ad_blk[:, 0:1].to_broadcast([P, P]),
                            id16_t[:])
                        adR = adR_ps
                        # one-hot S over all tiles of the block
                        S_all = sb.tile([P, tpb * P], f16, tag="S")
                        nc.vector.tensor_tensor(
                            out=S_all[:].rearrange("p (t v) -> p t v", v=P),
                            in0=rel_t[:, b * tpb:(b + 1) * tpb].rearrange(
                                "p (t u) -> p t u", u=1).to_broadcast(
                                [P, tpb, P]),
                            in1=io_b, op=Alu.is_equal)
                        # X[p,t,v] = ad[v] + as[p,t], as read straight from
                        # the gathered rows (col ASC), one op per quarter
                        X_all = sb.tile([P, tpb * P], f16, tag="X")
                        adR_b1 = adR[:].rearrange(
                            "p (u v) -> p u v", u=1).to_broadcast(
                            [P, tpq, P])
                        for q in range(NQ):
                            nc.vector.tensor_tensor(
                                out=X_all[:, q * tpq * P:
                                          (q + 1) * tpq * P].rearrange(
                                    "p (t v) -> p t v", v=P),
                                in0=adR_b1,
                                in1=gq3[q][:, j * tpq:(j + 1) * tpq,
                                           ASC:ASC + 1].to_broadcast(
                                    [P, tpq, P]),
                                op=Alu.add)
                        # exp(leakyrelu(x)) = max(exp(x), exp(0.2 x));
                        # both Exp -> no activation-table thrash
                        E2 = sb.tile([P, tpb * P], f16, tag="E2")
                        nc.scalar.activation(E2[:], X_all[:], Act.Exp,
                                             scale=0.2)
                        nc.scalar.activation(X_all[:], X_all[:], Act.Exp)
                        nc.vector.tensor_tensor(out=X_all[:], in0=X_all[:],
                                                in1=E2[:], op=Alu.max)
                        nc.vector.tensor_tensor(out=S_all[:], in0=S_all[:],
                                                in1=X_all[:], op=Alu.mult)
                        # self loop weight from the local row
                        xes = sbt.tile([P, 1], f32, tag="xes")
                        nc.vector.tensor_tensor(
                            out=xes[:], in0=sf[:, j * ROW + ASC:
                                               j * ROW + ASC + 1],
                            in1=sf[:, j * ROW + ADC:j * ROW + ADC + 1],
                            op=Alu.add)
                        e2s = sbt.tile([P, 1], f32, tag="e2s")
                        nc.scalar.activation(e2s[:], xes[:], Act.Exp,
                                             scale=0.2)
                        nc.scalar.activation(xes[:], xes[:], Act.Exp)
                        nc.vector.tensor_tensor(out=xes[:], in0=xes[:],
                                                in1=e2s[:], op=Alu.max)
                        exSs = sbt.tile([P, P], f16, tag="exSs")
                        nc.vector.tensor_tensor(
                            out=exSs[:], in0=id16_t[:],
                            in1=xes[:].to_broadcast([P, P]), op=Alu.mult)

                        acc = psacc.tile([P, RW], f32, tag="acc")
                        for q in range(NQ):
                            for i in range(tpq):
                                t = q * tpq + i
                                nc.tensor.matmul(
                                    acc[:],
                                    lhsT=S_all[:, t * P:(t + 1) * P],
                                    rhs=gq3[q][:, j * tpq + i:
                                               j * tpq + i + 1,
                                               0:RW].rearrange(
                                        "p c e -> p (c e)"),
                                    start=(t == 0), stop=False)
                        nc.tensor.matmul(
                            acc[:], lhsT=exSs[:],
                            rhs=sf[:, j * ROW:j * ROW + RW],
                            start=False, stop=True)

                        den = sbt.tile([P, 1], f32, tag="den")
                        nc.vector.tensor_scalar(
                            out=den[:], in0=acc[:, HID:HID + 1],
                            scalar1=1e-30, scalar2=None, op0=Alu.max)
                        rec = sbt.tile([P, 1], f32, tag="rec")
                        nc.vector.reciprocal(rec[:], den[:])
                        hv = sbt.tile([P, HID], f32, tag="hv")
                        nc.vector.tensor_tensor(
                            out=hv[:], in0=acc[:, 0:HID],
                            in1=rec[:].to_broadcast([P, HID]), op=Alu.mult)
                        if not is_last:
                            nc.vector.tensor_tensor(out=hv[:], in0=hv[:],
                                                    in1=b1_t[:], op=Alu.add)
                            nc.vector.tensor_scalar(out=hv[:], in0=hv[:],
                                                    scalar1=0.0, scalar2=None,
                                                    op0=Alu.max)
                            hv16 = sbt.tile([P, HID], f16, tag="hv16")
                            nc.vector.tensor_copy(hv16[:], hv[:])
                            hvT_ps = ps.tile([HID, P], f16, tag="pst")
                            nc.tensor.transpose(hvT_ps[:], hv16[:], id16_t[:])
                            hvT = sbt.tile([HID, P], f16, tag="hvT")
                            nc.vector.tensor_copy(hvT[:], hvT_ps[:])
                            t2T_ps = ps.tile([TW, P], f32, tag="pst")
                            nc.tensor.matmul(t2T_ps[:], lhsT=w2_t[:],
                                             rhs=hvT[:], start=True,
                                             stop=True)
                            t2T_sb = sbt.tile([TW, P], f16, tag="t2Tsb")
                            nc.vector.tensor_copy(t2T_sb[:], t2T_ps[:])
                            t2_ps = ps.tile([P, TW], f16, tag="pst")
                            nc.tensor.transpose(t2_ps[:], t2T_sb[:],
                                                id16_t[:TW, :TW])
                            t2_sb = sb.tile([P, TW], f16, tag="t2sb")
                            nc.vector.tensor_copy(t2_sb[:], t2_ps[:])
                            nc.vector.memset(t2_sb[:, ONEC:ONEC + 1], 1.0)
                            nc.sync.dma_start(
                                slice2[b * P:(b + 1) * P, 0:TW], t2_sb[:])
                        else:
                            nc.vector.tensor_tensor(out=hv[:], in0=hv[:],
                                                    in1=b2_t[:], op=Alu.add)
                            prhs = sbt.tile([P, HID + 1], f16, tag="prhs")
                            nc.vector.tensor_copy(prhs[:, 0:HID], hv[:])
                            nc.vector.tensor_copy(prhs[:, HID:HID + 1],
                                                  one16_t[:])
                            gid_col = sbt.tile([P, 1], f16, tag="gidcol")
                            nc.vector.tensor_copy(gid_col[:],
                                                  gid_t[:, b:b + 1])
                            Gh = sbt.tile([P, P], f16, tag="Gh")
                            nc.vector.tensor_tensor(
                                out=Gh[:],
                                in0=gid_col[:].to_broadcast([P, P]),
                                in1=io16_t[:], op=Alu.is_equal)
                            nc.tensor.matmul(pooled_ps[:], lhsT=Gh[:],
                                             rhs=prhs[:], start=(b == 0),
                                             stop=(b == nblk - 1))

            gat_layer(table1, slice1, is_last=False)
            for s in range(NQ):
                nc.gpsimd.collective_compute(
                    "AllGather", Alu.bypass,
                    replica_groups=[list(range(ncores))],
                    ins=[slice2[s * sub:(s + 1) * sub, :]],
                    outs=[table2[s * qrows:(s + 1) * qrows, :]],
                )
            gat_layer(table2, slice2, is_last=True)

            # ---- AllReduce pooled sums ----
            pooled_sb = sb.tile([P, HID + 1], f32, tag="pooledsb")
            nc.vector.tensor_copy(pooled_sb[:], pooled_ps[:])
            nc.sync.dma_start(pool_in[:, :], pooled_sb[:])
            nc.gpsimd.collective_compute(
                "AllReduce", Alu.add,
                replica_groups=[list(range(ncores))],
                ins=[pool_in.opt()], outs=[pool_out.opt()],
            )
            pl = sb.tile([P, HID + 1], f32, tag="pl")
            nc.sync.dma_start(pl[:], pool_out[:, :])

            cnt = sb.tile([P, 1], f32, tag="cnt")
            nc.vector.tensor_scalar(out=cnt[:], in0=pl[:, HID:HID + 1],
                                    scalar1=1.0, scalar2=None, op0=Alu.max)
            crec = sb.tile([P, 1], f32, tag="crec")
            nc.vector.reciprocal(crec[:], cnt[:])
            mean = sb.tile([P, HID], f32, tag="mean")
            nc.vector.tensor_tensor(out=mean[:], in0=pl[:, 0:HID],
                                    in1=crec[:].to_broadcast([P, HID]),
                                    op=Alu.mult)

            # MLP: z = relu(mean @ lin_w + lin_b); logits = z @ cls_w + cls_b
            lw_t = cst.tile([HID, HID // 2], f32)
            nc.sync.dma_start(lw_t[:], lw_d[:, :])
            lb_t = cst.tile([HID // 2, 1], f32)
            nc.sync.dma_start(lb_t[:], lb_d[:, :])
            cw_t = cst.tile([HID // 2, C], f32)
            nc.sync.dma_start(cw_t[:], cw_d[:, :])
            cb_t = cst.tile([C, 1], f32)
            nc.sync.dma_start(cb_t[:], cb_d[:, :])

            meanT_ps = ps.tile([HID, P], f32, tag="pst")
            nc.tensor.transpose(meanT_ps[:], mean[:], id32_t[:])
            meanT = sb.tile([HID, P], f32, tag="meanT")
            nc.vector.tensor_copy(meanT[:], meanT_ps[:])
            zT_ps = ps.tile([HID // 2, P], f32, tag="pst")
            nc.tensor.matmul(zT_ps[:], lhsT=lw_t[:], rhs=meanT[:],
                             start=True, stop=True)
            zT = sb.tile([HID // 2, P], f32, tag="zT")
            nc.scalar.activation(zT[:], zT_ps[:], Act.Relu, bias=lb_t[:])
            lgT_ps = ps.tile([C, P], f32, tag="pst")
            nc.tensor.matmul(lgT_ps[:], lhsT=cw_t[:], rhs=zT[:],
                             start=True, stop=True)
            lgT = sb.tile([C, P], f32, tag="lgT")
            nc.scalar.activation(lgT[:], lgT_ps[:], Act.Identity, bias=cb_t[:])
            lg_ps = ps.tile([P, C], f32, tag="pst")
            nc.tensor.transpose(lg_ps[:], lgT[:], id32_t[:C, :C])
            lg = sb.tile([P, C], f32, tag="lg")
            nc.vector.tensor_copy(lg[:], lg_ps[:])

            mx = sb.tile([P, 1], f32, tag="mx")
            nc.vector.tensor_reduce(mx[:], lg[:], axis=mybir.AxisListType.X,
                                    op=Alu.max)
            sh = sb.tile([P, C], f32, tag="sh")
            nc.vector.tensor_tensor(out=sh[:], in0=lg[:],
                                    in1=mx[:].to_broadcast([P, C]),
                                    op=Alu.subtract)
            exs = sb.tile([P, C], f32, tag="exs")
            se = sb.tile([P, 1], f32, tag="se")
            nc.scalar.activation(exs[:], sh[:], Act.Exp, accum_out=se[:])
            lse = sb.tile([P, 1], f32, tag="lse")
            nc.scalar.activation(lse[:], se[:], Act.Ln)
            res = sb.tile([P, C], f32, tag="res")
            nc.vector.tensor_tensor(out=res[:], in0=sh[:],
                                    in1=lse[:].to_broadcast([P, C]),
                                    op=Alu.subtract)
            nc.sync.dma_start(out_d[:, :], res[:])

    nc.compile()
    return nc


def run_gnn(inputs, ncores=8, trace=False):
    from concourse.bass_utils import run_bass_kernel_spmd

    x = np.asarray(inputs["x"], np.float32)
    edge_index = np.asarray(inputs["edge_index"])
    batch = np.asarray(inputs["batch"])
    W1 = np.asarray(inputs["W1"], np.float32)
    W2 = np.asarray(inputs["W2"], np.float32)
    hd = _build_host_data(
        x, edge_index, batch, W1,
        np.asarray(inputs["a_src1"], np.float32),
        np.asarray(inputs["a_dst1"], np.float32),
        W2,
        np.asarray(inputs["a_src2"], np.float32),
        np.asarray(inputs["a_dst2"], np.float32),
        ncores)

    N, F_IN = x.shape
    G = 128  # number of graphs == P (pooling one-hot relies on this)
    C = np.asarray(inputs["cls_w"]).shape[1]

    nc = _build_program(ncores, hd["nblk"], hd["tpq"], F_IN, G, C,
                        hd["V"], hd["qrows"], hd["nchunk"], hd["chunk"])

    iota16 = np.tile(np.arange(P, dtype=np.float16)[None, :], (P, 1))
    ident16 = np.eye(P, dtype=np.float16)
    ident32 = np.eye(P, dtype=np.float32)
    ones16 = np.ones((P, 1), np.float16)
    b1rep = np.tile(np.asarray(inputs["b1"], np.float32)[None, :], (P, 1))
    b2rep = np.tile(np.asarray(inputs["b2"], np.float32)[None, :], (P, 1))

    in_maps = []
    for c in range(ncores):
        in_maps.append({
            "xT": hd["xT_all"][c],
            "idx": hd["idx_all"][c],
            "rel": hd["rel_all"][c],
            "gid": hd["gid_all"][c],
            "w1aug": hd["W1aug"],
            "w2aug": hd["W2aug"],
            "b1rep": b1rep,
            "b2rep": b2rep,
            "lin_w": np.asarray(inputs["lin_w"], np.float32),
            "lin_b": np.asarray(inputs["lin_b"], np.float32)[:, None],
            "cls_w": np.asarray(inputs["cls_w"], np.float32),
            "cls_b": np.asarray(inputs["cls_b"], np.float32)[:, None],
            "iota16": iota16,
            "ident16": ident16,
            "ident32": ident32,
            "ones16": ones16,
        })

    res = run_bass_kernel_spmd(nc, in_maps, core_ids=list(range(ncores)),
                               trace=trace)
    out = res.results[0]["out"]
    return out, res


def kernel(**inputs):
    out, _ = run_gnn(inputs, ncores=8)
    return out.astype(np.float32)


# revision 31
# speedup vs baseline: 3.9519x; 1.0347x over previous
"""GAT (2-layer) + global mean pool + MLP + log_softmax on 8 Trainium2 cores.

Strategy (dst-sharded message passing, bulk-gather edition):
  - Nodes partitioned across 8 cores; per-core node tables
    ([h@W | 1 | as | ad] as 128-col f16 rows = 256B) are computed shard-wise
    and replicated via AllGather into DRAM tables.
  - Per-edge h[src] rows are fetched with Pool-engine dma_gather (<=1024
    indices per call - the ucode limit), one call per (dst-block, quarter),
    rotated over 4 SWDGE queues (descriptor generation parallelizes ~3.6x
    across queues).  The table is addressed in 4 quarters so indices fit
    int16.
  - Local nodes are bin-packed into blocks balancing per-(block, quarter)
    edge counts, so every bucket fits tpq tiles of 128 edges with minimal
    padding.  Self loops bypass the gather entirely (local rows, identity
    one-hot).
  - Per-edge attention weights use exp(leakyrelu(x)) = max(exp(x),
    exp(0.2x)): two Exp passes on the otherwise-idle scalar engine (table
    pinned to Exp), so the vector engine only builds the one-hot, one add
    and one max per block.
  - Aggregation is a PSUM-accumulated one-hot matmul per 128-edge tile with
    the softmax denominator riding along as the table's constant-1 column.
  - Graph pooling = one-hot matmul + AllReduce; tiny MLP + log_softmax run
    redundantly on every core.
"""

import sys

sys.path.insert(0, "/opt/trn_rl_repo")

import numpy as np

P = 128
NQ = 4          # src-quarters (int16 index range per dma_gather)
ROW = 128       # table row width in f16 elems (256B, dma_gather granularity)
HID = 64
ONEC = HID      # constant-1 column (denominator rides the matmul)
ASC = HID + 1   # alpha_src column
ADC = HID + 2   # alpha_dst column
TW = HID + 3    # populated row prefix
RW = HID + 1    # rhs width for the aggregation matmul: [h | 1]
CHUNK = 10      # dst blocks per gather chunk


def _pack_blocks(deg_q, nblk, cap):
    """First-fit-decreasing bin packing: assign nodes (rows of deg_q
    [npc, NQ]) to nblk bins with <= P nodes per bin and per-quarter edge
    count <= cap.  Returns pos[npc] (slot b*P + i) or None."""
    npc = deg_q.shape[0]
    order = np.argsort(-deg_q.sum(1), kind="stable")
    rem = np.full((nblk, NQ), cap, np.int64)
    cnt = np.zeros(nblk, np.int64)
    pos = np.empty(npc, np.int64)
    for l in order:
        ok = (cnt < P) & (rem >= deg_q[l]).all(1)
        b = int(np.argmax(ok))
        if not ok[b]:
            return None
        pos[l] = b * P + cnt[b]
        cnt[b] += 1
        rem[b] -= deg_q[l]
    return pos


def _build_host_data(x, edge_index, batch, W1, a_src1, a_dst1, W2, a_src2,
                     a_dst2, ncores):
    """Pure-integer/graph preprocessing + augmented weights (host side)."""
    N, F_IN = x.shape
    assert N % ncores == 0 and ncores % NQ == 0
    npc = N // ncores
    assert npc % NQ == 0
    npg = npc // NQ  # nodes per (core, group)

    # self loops are handled separately on-device (local rows, no gather)
    src = np.asarray(edge_index[0])
    dst = np.asarray(edge_index[1])
    score = (src // npc).astype(np.int64)
    dcore = (dst // npc).astype(np.int64)
    sloc = (src % npc).astype(np.int64)
    dloc = (dst % npc).astype(np.int64)

    # the table is laid out in NQ segments; segment q holds every core's
    # q-th node group, so a node's int16-index quarter equals its group
    # (known before packing) and the table AllGather can be pipelined as
    # NQ sub-collectives
    q_of = sloc // npg

    # per-node quarter in-degree, then degree-balanced packing into blocks
    deg = np.zeros((ncores, npc, NQ), np.int64)
    np.add.at(deg, (dcore, dloc, q_of), 1)

    # choose (tiles-per-quarter cap, per-group block count) minimizing total
    # tiles; extra blocks buy packing slack that lets a lower cap succeed
    nb4_base = -(-npg // P)
    pos_all = np.empty((ncores, npc), np.int64)
    tpq = None
    opts = [(cap, nb4)
            for cap in range(1, 9)
            for nb4 in (nb4_base, nb4_base + 1, nb4_base + 2, nb4_base + 3)
            if ncores * nb4 * P <= 32767 or nb4 == nb4_base]
    opts.sort(key=lambda o: o[1] * (NQ * o[0] + 1))
    qload = deg.sum(1)  # [ncores, NQ]
    gload = np.zeros((ncores, NQ, NQ), np.int64)  # [core, dst group, src q]
    np.add.at(gload, (dcore, dloc // npg, q_of), 1)
    for cap_tiles, nb4 in opts:
        if gload.max() > nb4 * cap_tiles * P:
            continue
        ok = True
        for c in range(ncores):
            for g in range(NQ):
                ids = np.arange(g * npg, (g + 1) * npg)
                pos = _pack_blocks(deg[c][ids], nb4, cap_tiles * P)
                if pos is None:
                    ok = False
                    break
                pos_all[c, ids] = g * nb4 * P + pos
            if not ok:
                break
        if ok:
            tpq = cap_tiles
            break
    assert tpq is not None
    assert tpq * P <= 1024  # dma_gather ucode faults above 1024 indices
    nblk = NQ * nb4
    npc_pad = nblk * P
    sub = nb4 * P           # rows per (core, segment)
    V = ncores * npc_pad
    qrows = V // NQ
    assert qrows <= 32767
    chunk = nb4
    nchunk = NQ
    tpb = NQ * tpq

    spos = pos_all[score, sloc]
    srow = (spos // sub) * qrows + score * sub + spos % sub
    qoff = (srow % qrows).astype(np.int64)
    dpos = pos_all[dcore, dloc]
    blk = dpos // P
    rel = dpos % P

    cnt = np.zeros((ncores, nblk, NQ), np.int64)
    np.add.at(cnt, (dcore, blk, q_of), 1)
    tpq = int(max(1, -(-cnt.max() // P)))
    tpb = NQ * tpq

    call_idx = np.zeros((ncores, NQ, nblk, tpq * P), np.int16)
    rel_all = np.full((ncores, P, nblk * tpb), -1.0, np.float16)

    order = np.lexsort((q_of, blk, dcore))
    so_q, so_b, so_c = q_of[order], blk[order], dcore[order]
    so_qoff, so_rel = qoff[order], rel[order]
    key = (so_c * nblk + so_b) * NQ + so_q
    start = np.searchsorted(key, np.arange(ncores * nblk * NQ), side="left")
    pos = np.arange(len(key)) - start[key]
    call_idx[so_c, so_q, so_b, pos] = so_qoff.astype(np.int16)
    rel_col = so_b * tpb + so_q * tpq + pos // P
    rel_all[so_c, pos % P, rel_col] = so_rel.astype(np.float16)

    # wrap indices for the gpsimd cores: idx j lives at [p % 16 == j % 16,
    # j // 16], replicated across the 8 groups of 16 partitions
    ci = call_idx.reshape(ncores, NQ, nblk, -1, 16)
    ci = np.transpose(ci, (0, 1, 2, 4, 3))
    ci = np.broadcast_to(ci[:, :, :, None, :, :],
                         (ncores, NQ, nblk, 8, 16, tpq * P // 16))
    idx_all = ci.reshape(ncores, NQ, nblk, P, -1)
    idx_all = np.transpose(idx_all, (0, 3, 1, 2, 4)).reshape(ncores, P, -1)
    idx_all = np.ascontiguousarray(idx_all)

    gid_all = np.full((ncores, P, nblk), -1.0, np.float32)
    xT_all = np.zeros((ncores, F_IN, npc_pad), np.float16)
    for c in range(ncores):
        ids = np.arange(npc)
        gg = np.full(npc_pad, -1.0, np.float32)
        gg[pos_all[c]] = batch[ids + c * npc].astype(np.float32)
        gid_all[c] = gg.reshape(nblk, P).T
        xT_all[c][:, pos_all[c]] = x[c * npc:(c + 1) * npc].T.astype(
            np.float16)

    def aug(W, a_s, a_d):
        w = np.zeros((W.shape[0], TW), np.float32)
        w[:, :HID] = W
        w[:, ASC] = W @ a_s
        w[:, ADC] = W @ a_d
        return w.astype(np.float16)

    return dict(npc=npc, nblk=nblk, npc_pad=npc_pad, tpq=tpq, tpb=tpb, V=V,
                chunk=chunk, qrows=qrows, nchunk=nchunk, idx_all=idx_all,
                rel_all=rel_all, gid_all=gid_all, xT_all=xT_all,
                W1aug=aug(W1, a_src1, a_dst1), W2aug=aug(W2, a_src2, a_dst2))


def _build_program(ncores, nblk, tpq, F_IN, G, C, V, qrows, nchunk, chunk):
    import concourse.bass as bass
    import concourse.bacc as bacc
    import concourse.tile as tile
    from concourse import mybir

    tpb = NQ * tpq
    npc_pad = nblk * P
    nidx = tpq * P                    # indices per (block, quarter) gather
    idxw = nidx // 16                 # idx cols per call (int16, wrapped)

    nc = bacc.Bacc("TRN2", target_bir_lowering=False, debug=False,
                   num_devices=ncores, num_swdge_queues=4)
    f32, f16, i16 = mybir.dt.float32, mybir.dt.float16, mybir.dt.int16
    Alu = mybir.AluOpType
    Act = mybir.ActivationFunctionType

    ein = lambda n, s, d: nc.dram_tensor(n, s, d, kind="ExternalInput")
    xT_d = ein("xT", [F_IN, npc_pad], f16)
    idx_d = ein("idx", [P, NQ * nblk * idxw], i16)
    rel_d = ein("rel", [P, nblk * tpb], f16)
    gid_d = ein("gid", [P, nblk], f32)
    w1_d = ein("w1aug", [F_IN, TW], f16)
    w2_d = ein("w2aug", [HID, TW], f16)
    b1_d = ein("b1rep", [P, HID], f32)
    b2_d = ein("b2rep", [P, HID], f32)
    lw_d = ein("lin_w", [HID, HID // 2], f32)
    lb_d = ein("lin_b", [HID // 2, 1], f32)
    cw_d = ein("cls_w", [HID // 2, C], f32)
    cb_d = ein("cls_b", [C, 1], f32)
    io16_d = ein("iota16", [P, P], f16)
    id16_d = ein("ident16", [P, P], f16)
    id32_d = ein("ident32", [P, P], f32)
    one16_d = ein("ones16", [P, 1], f16)
    out_d = nc.dram_tensor("out", [G, C], f32, kind="ExternalOutput")

    table1 = nc.dram_tensor("table1", [V, ROW], f16, kind="Internal")
    table2 = nc.dram_tensor("table2", [V, ROW], f16, kind="Internal")

    with tile.TileContext(nc) as tc:
        with (
            tc.tile_pool(name="cst", bufs=1) as cst,
            tc.tile_pool(name="sb", bufs=4) as sb,
            tc.tile_pool(name="sbt", bufs=6) as sbt,
            tc.tile_pool(name="gat", bufs=6) as gat,
            tc.tile_pool(name="ps", bufs=3, space="PSUM") as ps,
            tc.tile_pool(name="psa", bufs=2, space="PSUM") as psa,
            tc.tile_pool(name="psacc", bufs=2, space="PSUM") as psacc,
            tc.tile_pool(name="pspool", bufs=1, space="PSUM") as pspool,
            tc.tile_pool(name="dram", bufs=1, space="DRAM") as dram,
        ):
            # ---- constants ----
            rel_t = cst.tile([P, nblk * tpb], f16)
            nc.sync.dma_start(rel_t[:], rel_d[:, :])
            gid_t = cst.tile([P, nblk], f32)
            nc.sync.dma_start(gid_t[:], gid_d[:, :])
            w1_t = cst.tile([F_IN, TW], f16)
            nc.sync.dma_start(w1_t[:], w1_d[:, :])
            w2_t = cst.tile([HID, TW], f16)
            nc.sync.dma_start(w2_t[:], w2_d[:, :])
            b1_t = cst.tile([P, HID], f32)
            nc.sync.dma_start(b1_t[:], b1_d[:, :])
            b2_t = cst.tile([P, HID], f32)
            nc.sync.dma_start(b2_t[:], b2_d[:, :])
            io16_t = cst.tile([P, P], f16)
            nc.sync.dma_start(io16_t[:], io16_d[:, :])
            id16_t = cst.tile([P, P], f16)
            nc.sync.dma_start(id16_t[:], id16_d[:, :])
            id32_t = cst.tile([P, P], f32)
            nc.sync.dma_start(id32_t[:], id32_d[:, :])
            one16_t = cst.tile([P, 1], f16)
            nc.sync.dma_start(one16_t[:], one16_d[:, :])
            xT_t = cst.tile([F_IN, npc_pad], f16)
            nc.sync.dma_start(xT_t[:], xT_d[:, :])
            idx_t = cst.tile([P, NQ * nblk * idxw], i16)
            nc.sync.dma_start(idx_t[:], idx_d[:, :])

            slice1 = dram.tile([npc_pad, ROW], f16)
            slice2 = dram.tile([npc_pad, ROW], f16)
            pool_in = dram.tile([P, HID + 1], f32)
            pool_out = dram.tile([P, HID + 1], f32)

            pooled_ps = pspool.tile([P, HID + 1], f32)

            io_b = io16_t[:].rearrange("p (u v) -> p u v", u=1).to_broadcast(
                [P, tpb, P])
            qcall = [0]

            # ---- phase 0: slice1 rows = [x@W1 | 1 | as1 | ad1] ----
            for b in range(nblk):
                t1T_ps = ps.tile([TW, P], f32, tag="pst")
                nc.tensor.matmul(t1T_ps[:], lhsT=w1_t[:],
                                 rhs=xT_t[:, b * P:(b + 1) * P],
                                 start=True, stop=True)
                t1T_sb = sb.tile([TW, P], f16, tag="t1Tsb")
                nc.vector.tensor_copy(t1T_sb[:], t1T_ps[:])
                t1_ps = ps.tile([P, TW], f16, tag="pst")
                nc.tensor.transpose(t1_ps[:], t1T_sb[:], id16_t[:TW, :TW])
                t1_sb = sb.tile([P, TW], f16, tag="t1sb")
                nc.vector.tensor_copy(t1_sb[:], t1_ps[:])
                nc.vector.memset(t1_sb[:, ONEC:ONEC + 1], 1.0)
                nc.sync.dma_start(slice1[b * P:(b + 1) * P, 0:TW], t1_sb[:])

            sub = npc_pad // NQ
            for s in range(NQ):
                nc.gpsimd.collective_compute(
                    "AllGather", Alu.bypass,
                    replica_groups=[list(range(ncores))],
                    ins=[slice1[s * sub:(s + 1) * sub, :]],
                    outs=[table1[s * qrows:(s + 1) * qrows, :]],
                )

            def gat_layer(table_h, slice_ap, is_last):
                lname = "L2" if is_last else "L1"
                # ad[dst] for local nodes: ad_grid[p, b] = slice[b*128+p, ADC]
                ad_grid = cst.tile([P, nblk], f16, name=f"adg{lname}")
                nc.sync.dma_start(
                    ad_grid[:],
                    slice_ap[:, ADC:ADC + 1].rearrange(
                        "(b p) c -> p (b c)", p=P),
                )
                # blocks per gather call, bounded by the 1024-index ucode
                # limit; gathers pipeline at (bpc-blocks, quarter) grain
                bpc = max(1, 1024 // nidx)
                while nblk % bpc:
                    bpc -= 1
                for bp in range(nblk // bpc):
                    gq3 = []
                    for q in range(NQ):
                        g = gat.tile([P, bpc * tpq * ROW], f16, tag=f"g{q}")
                        g3 = g[:].rearrange("p (c e) -> p c e", e=ROW)
                        nc.gpsimd.dma_gather(
                            out_ap=g3,
                            in_ap=table_h[q * qrows:(q + 1) * qrows, :],
                            idxs_ap=idx_t[:, (q * nblk + bp * bpc) * idxw:
                                          (q * nblk + (bp + 1) * bpc) * idxw],
                            num_idxs=nidx * bpc,
                            num_idxs_reg=nidx * bpc,
                            elem_size=ROW,
                            queue_num=qcall[0] % 4,
                        )
                        qcall[0] += 1
                        gq3.append(g3)
                    # self-loop rows of these blocks (local, seq DMA)
                    sf = gat.tile([P, bpc * ROW], f16, tag="self")
                    nc.sync.dma_start(
                        sf[:].rearrange("p (b e) -> p b e", e=ROW),
                        slice_ap[bp * bpc * P:(bp + 1) * bpc * P,
                                 :].rearrange("(b p) e -> p b e", p=P))

                    for j in range(bpc):
                        b = bp * bpc + j
                        # adR[p, v] = ad of dst v in this block
                        adR_ps = psa.tile([P, P], f16, tag="adps")
                        nc.tensor.transpose(
                            adR_ps[:],
                            ad_grid[:, b:b + 1].to_broadcast([P, P]),
                            id16_t[:])
                        adR = adR_ps
                        # one-hot S over all tiles of the block
                        S_all = sb.tile([P, tpb * P], f16, tag="S")
                        nc.vector.tensor_tensor(
                            out=S_all[:].rearrange("p (t v) -> p t v", v=P),
                            in0=rel_t[:, b * tpb:(b + 1) * tpb].rearrange(
                                "p (t u) -> p t u", u=1).to_broadcast(
                                [P, tpb, P]),
                            in1=io_b, op=Alu.is_equal)
                        # X[p,t,v] = ad[v] + as[p,t], as read straight from
                        # the gathered rows (col ASC), one op per quarter
                        X_all = sb.tile([P, tpb * P], f16, tag="X")
                        adR_b1 = adR[:].rearrange(
                            "p (u v) -> p u v", u=1).to_broadcast(
                            [P, tpq, P])
                        for q in range(NQ):
                            nc.vector.tensor_tensor(
                                out=X_all[:, q * tpq * P:
                                          (q + 1) * tpq * P].rearrange(
                                    "p (t v) -> p t v", v=P),
                                in0=adR_b1,
                                in1=gq3[q][:, j * tpq:(j + 1) * tpq,
                                           ASC:ASC + 1].to_broadcast(
                                    [P, tpq, P]),
                                op=Alu.add)
                        # exp(leakyrelu(x)) = max(exp(x), exp(0.2 x));
                        # both Exp -> no activation-table thrash
                        E2 = sb.tile([P, tpb * P], f16, tag="E2")
                        nc.scalar.activation(E2[:], X_all[:], Act.Exp,
                                             scale=0.2)
                        nc.scalar.activation(X_all[:], X_all[:], Act.Exp)
                        nc.vector.tensor_tensor(out=X_all[:], in0=X_all[:],
                                                in1=E2[:], op=Alu.max)
                        nc.vector.tensor_tensor(out=S_all[:], in0=S_all[:],
                                                in1=X_all[:], op=Alu.mult)
                        # self loop weight from the local row
                        xes = sbt.tile([P, 1], f32, tag="xes")
                        nc.vector.tensor_tensor(
                            out=xes[:], in0=sf[:, j * ROW + ASC:
                                               j * ROW + ASC + 1],
                            in1=sf[:, j * ROW + ADC:j * ROW + ADC + 1],
                            op=Alu.add)
                        e2s = sbt.tile([P, 1], f32, tag="e2s")
                        nc.scalar.activation(e2s[:], xes[:], Act.Exp,
                                             scale=0.2)
                        nc.scalar.activation(xes[:], xes[:], Act.Exp)
                        nc.vector.tensor_tensor(out=xes[:], in0=xes[:],
                                                in1=e2s[:], op=Alu.max)
                        exSs = sbt.tile([P, P], f16, tag="exSs")
                        nc.vector.tensor_tensor(
                            out=exSs[:], in0=id16_t[:],
                            in1=xes[:].to_broadcast([P, P]), op=Alu.mult)

                        acc = psacc.tile([P, RW], f32, tag="acc")
                        for q in range(NQ):
                            for i in range(tpq):
                                t = q * tpq + i
                                nc.tensor.matmul(
                                    acc[:],
                                    lhsT=S_all[:, t * P:(t + 1) * P],
                                    rhs=gq3[q][:, j * tpq + i:
                                               j * tpq + i + 1,
                                               0:RW].rearrange(
                                        "p c e -> p (c e)"),
                                    start=(t == 0), stop=False)
                        nc.tensor.matmul(
                            acc[:], lhsT=exSs[:],
                            rhs=sf[:, j * ROW:j * ROW + RW],
                            start=False, stop=True)

                        den = sbt.tile([P, 1], f32, tag="den")
                        nc.vector.tensor_scalar(
                            out=den[:], in0=acc[:, HID:HID + 1],
                            scalar1=1e-30, scalar2=None, op0=Alu.max)
                        rec = sbt.tile([P, 1], f32, tag="rec")
                        nc.vector.reciprocal(rec[:], den[:])
                        hv = sbt.tile([P, HID], f32, tag="hv")
                        nc.vector.tensor_tensor(
                            out=hv[:], in0=acc[:, 0:HID],
                            in1=rec[:].to_broadcast([P, HID]), op=Alu.mult)
                        if not is_last:
                            nc.vector.tensor_tensor(out=hv[:], in0=hv[:],
                                                    in1=b1_t[:], op=Alu.add)
                            nc.vector.tensor_scalar(out=hv[:], in0=hv[:],
                                                    scalar1=0.0, scalar2=None,
                                                    op0=Alu.max)
                            hv16 = sbt.tile([P, HID], f16, tag="hv16")
                            nc.vector.tensor_copy(hv16[:], hv[:])
                            hvT_ps = ps.tile([HID, P], f16, tag="pst")
                            nc.tensor.transpose(hvT_ps[:], hv16[:], id16_t[:])
                            hvT = sbt.tile([HID, P], f16, tag="hvT")
                            nc.vector.tensor_copy(hvT[:], hvT_ps[:])
                            t2T_ps = ps.tile([TW, P], f32, tag="pst")
                            nc.tensor.matmul(t2T_ps[:], lhsT=w2_t[:],
                                             rhs=hvT[:], start=True,
                                             stop=True)
                            t2T_sb = sbt.tile([TW, P], f16, tag="t2Tsb")
                            nc.vector.tensor_copy(t2T_sb[:], t2T_ps[:])
                            t2_ps = ps.tile([P, TW], f16, tag="pst")
                            nc.tensor.transpose(t2_ps[:], t2T_sb[:],
                                                id16_t[:TW, :TW])
                            t2_sb = sb.tile([P, TW], f16, tag="t2sb")
                            nc.vector.tensor_copy(t2_sb[:], t2_ps[:])
                            nc.vector.memset(t2_sb[:, ONEC:ONEC + 1], 1.0)
                            nc.sync.dma_start(
                                slice2[b * P:(b + 1) * P, 0:TW], t2_sb[:])
                        else:
                            nc.vector.tensor_tensor(out=hv[:], in0=hv[:],
                                                    in1=b2_t[:], op=Alu.add)
                            prhs = sbt.tile([P, HID + 1], f16, tag="prhs")
                            nc.vector.tensor_copy(prhs[:, 0:HID], hv[:])
                            nc.vector.tensor_copy(prhs[:, HID:HID + 1],
                                                  one16_t[:])
                            gid_col = sbt.tile([P, 1], f16, tag="gidcol")
                            nc.vector.tensor_copy(gid_col[:],
                                                  gid_t[:, b:b + 1])
                            Gh = sbt.tile([P, P], f16, tag="Gh")
                            nc.vector.tensor_tensor(
                                out=Gh[:],
                                in0=gid_col[:].to_broadcast([P, P]),
                                in1=io16_t[:], op=Alu.is_equal)
                            nc.tensor.matmul(pooled_ps[:], lhsT=Gh[:],
                                             rhs=prhs[:], start=(b == 0),
                                             stop=(b == nblk - 1))

            gat_layer(table1, slice1, is_last=False)
            for s in range(NQ):
                nc.gpsimd.collective_compute(
                    "AllGather", Alu.bypass,
                    replica_groups=[list(range(ncores))],
                    ins=[slice2[s * sub:(s + 1) * sub, :]],
                    outs=[table2[s * qrows:(s + 1) * qrows, :]],
                )
            gat_layer(table2, slice2, is_last=True)

            # ---- AllReduce pooled sums ----
            pooled_sb = sb.tile([P, HID + 1], f32, tag="pooledsb")
            nc.vector.tensor_copy(pooled_sb[:], pooled_ps[:])
            nc.sync.dma_start(pool_in[:, :], pooled_sb[:])
            nc.gpsimd.collective_compute(
                "AllReduce", Alu.add,
                replica_groups=[list(range(ncores))],
                ins=[pool_in.opt()], outs=[pool_out.opt()],
            )
            pl = sb.tile([P, HID + 1], f32, tag="pl")
            nc.sync.dma_start(pl[:], pool_out[:, :])

            cnt = sb.tile([P, 1], f32, tag="cnt")
            nc.vector.tensor_scalar(out=cnt[:], in0=pl[:, HID:HID + 1],
                                    scalar1=1.0, scalar2=None, op0=Alu.max)
            crec = sb.tile([P, 1], f32, tag="crec")
            nc.vector.reciprocal(crec[:], cnt[:])
            mean = sb.tile([P, HID], f32, tag="mean")
            nc.vector.tensor_tensor(out=mean[:], in0=pl[:, 0:HID],
                                    in1=crec[:].to_broadcast([P, HID]),
                                    op=Alu.mult)

            # MLP: z = relu(mean @ lin_w + lin_b); logits = z @ cls_w + cls_b
            lw_t = cst.tile([HID, HID // 2], f32)
            nc.sync.dma_start(lw_t[:], lw_d[:, :])
            lb_t = cst.tile([HID // 2, 1], f32)
            nc.sync.dma_start(lb_t[:], lb_d[:, :])
            cw_t = cst.tile([HID // 2, C], f32)
            nc.sync.dma_start(cw_t[:], cw_d[:, :])
            cb_t = cst.tile([C, 1], f32)
            nc.sync.dma_start(cb_t[:], cb_d[:, :])

            meanT_ps = ps.tile([HID, P], f32, tag="pst")
            nc.tensor.transpose(meanT_ps[:], mean[:], id32_t[:])
            meanT = sb.tile([HID, P], f32, tag="meanT")
            nc.vector.tensor_copy(meanT[:], meanT_ps[:])
            zT_ps = ps.tile([HID // 2, P], f32, tag="pst")
            nc.tensor.matmul(zT_ps[:], lhsT=lw_t[:], rhs=meanT[:],
                             start=True, stop=True)
            zT = sb.tile([HID // 2, P], f32, tag="zT")
            nc.scalar.activation(zT[:], zT_ps[:], Act.Relu, bias=lb_t[:])
            lgT_ps = ps.tile([C, P], f32, tag="pst")
            nc.tensor.matmul(lgT_ps[:], lhsT=cw_t[:], rhs=zT[:],
                             start=True, stop=True)
            lgT = sb.tile([C, P], f32, tag="lgT")
            nc.scalar.activation(lgT[:], lgT_ps[:], Act.Identity, bias=cb_t[:])
            lg_ps = ps.tile([P, C], f32, tag="pst")
            nc.tensor.transpose(lg_ps[:], lgT[:], id32_t[:C, :C])
            lg = sb.tile([P, C], f32, tag="lg")
            nc.vector.tensor_copy(lg[:], lg_ps[:])

            mx = sb.tile([P, 1], f32, tag="mx")
            nc.vector.tensor_reduce(mx[:], lg[:], axis=mybir.AxisListType.X,
                                    op=Alu.max)
            sh = sb.tile([P, C], f32, tag="sh")
            nc.vector.tensor_tensor(out=sh[:], in0=lg[:],
                                    in1=mx[:].to_broadcast([P, C]),
                                    op=Alu.subtract)
            exs = sb.tile([P, C], f32, tag="exs")
            se = sb.tile([P, 1], f32, tag="se")
            nc.scalar.activation(exs[:], sh[:], Act.Exp, accum_out=se[:])
            lse = sb.tile([P, 1], f32, tag="lse")
            nc.scalar.activation(lse[:], se[:], Act.Ln)
            res = sb.tile([P, C], f32, tag="res")
            nc.vector.tensor_tensor(out=res[:], in0=sh[:],
                                    in1=lse[:].to_broadcast([P, C]),
                                    op=Alu.subtract)
            nc.sync.dma_start(out_d[:, :], res[:])

    nc.compile()
    return nc


def run_gnn(inputs, ncores=8, trace=False):
    from concourse.bass_utils import run_bass_kernel_spmd

    x = np.asarray(inputs["x"], np.float32)
    edge_index = np.asarray(inputs["edge_index"])
    batch = np.asarray(inputs["batch"])
    W1 = np.asarray(inputs["W1"], np.float32)
    W2 = np.asarray(inputs["W2"], np.float32)
    hd = _build_host_data(
        x, edge_index, batch, W1,
        np.asarray(inputs["a_src1"], np.float32),
        np.asarray(inputs["a_dst1"], np.float32),
        W2,
        np.asarray(inputs["a_src2"], np.float32),
        np.asarray(inputs["a_dst2"], np.float32),
        ncores)

    N, F_IN = x.shape
    G = 128  # number of graphs == P (pooling one-hot relies on this)
    C = np.asarray(inputs["cls_w"]).shape[1]

    nc = _build_program(ncores, hd["nblk"], hd["tpq"], F_IN, G, C,
                        hd["V"], hd["qrows"], hd["nchunk"], hd["chunk"])

    iota16 = np.tile(np.arange(P, dtype=np.float16)[None, :], (P, 1))
    ident16 = np.eye(P, dtype=np.float16)
    ident32 = np.eye(P, dtype=np.float32)
    ones16 = np.ones((P, 1), np.float16)
    b1rep = np.tile(np.asarray(inputs["b1"], np.float32)[None, :], (P, 1))
    b2rep = np.tile(np.asarray(inputs["b2"], np.float32)[None, :], (P, 1))

    in_maps = []
    for c in range(ncores):
        in_maps.append({
            "xT": hd["xT_all"][c],
            "idx": hd["idx_all"][c],
            "rel": hd["rel_all"][c],
            "gid": hd["gid_all"][c],
            "w1aug": hd["W1aug"],
            "w2aug": hd["W2aug"],
            "b1rep": b1rep,
            "b2rep": b2rep,
            "lin_w": np.asarray(inputs["lin_w"], np.float32),
            "lin_b": np.asarray(inputs["lin_b"], np.float32)[:, None],
            "cls_w": np.asarray(inputs["cls_w"], np.float32),
            "cls_b": np.asarray(inputs["cls_b"], np.float32)[:, None],
            "iota16": iota16,
            "ident16": ident16,
            "ident32": ident32,
            "ones16": ones16,
        })

    res = run_bass_kernel_spmd(nc, in_maps, core_ids=list(range(ncores)),
                               trace=trace)
    out = res.results[0]["out"]
    return out, res


def kernel(**inputs):
    out, _ = run_gnn(inputs, ncores=8)
    return out.astype(np.float32)
